# revision 28
# baseline (speedup 1.0000x reference)
"""DCRNCognition Trainium2 kernel — fp8 DoubleRow edition.

Self-contained: builds a Bass/Tile SPMD program for 8 NeuronCores, shards the
batch (conversation) axis across cores, runs via run_bass_kernel_spmd, and
gathers the valid positions on the host.

Math restructuring (identical to the verified baseline, rel err ~9e-7 in f32):
  - fc layer folded into step-1 LSTM gates; step-1 f-gate/c-init dead
  - step-2: gates2 = hs1 @ Wh.T + r1 @ Wr.T  (Wh = 0.5*(w_ih[:, :D]+w_hh))
  - softmax normalization deferred to r:  r = (X^T A) * (1/sum_u A)
  - sigmoid via tanh; h,c carried scaled by 2 (hs=2h, cs=2c)

Precision plan (validated on host: rel err ~6e-4 vs the 2e-2 gate):
  - ALL matmuls fp8 e4m3 with DoubleRow perf mode, fp32 PSUM accum.
  - weights pre-scaled by a power of 2 into fp8 range on the host; the
    compensation folds into the free activation `scale` operands.
  - g-gate weight rows doubled on host so every gate activation shares
    scale=0.5 -> one ACT instruction per gate pair.
  - masking without exp bias: invalid bank rows (u >= len) are zeroed on
    the host in the u-major layout (kills their r contribution) and the
    softmax denominator contracts A against a per-conversation 0/1 mask
    as the matmul stationary (kills them in the sum).
  - logits computed transposed ([t,C]): log-softmax along the free axis,
    one batched Ln at the end -> 2 ACT table loads total.

Scheduling: 4-stage software pipeline (F=gates1+cell1, B1=attention1,
B2=gates2+cell2, B3=attention2+logits) with a 3-conversation skew so every
engine queue always holds ready work. PSUM: two 4-bank pools (gates /
attention+logits).
"""
import os
import sys
sys.path.insert(0, '/opt/trn_rl_repo')

# run_bass_kernel_spmd executes through jax/PJRT on the axon-tunneled
# NeuronCores; a JAX_PLATFORMS=cpu pin would hide them.
if os.environ.get('JAX_PLATFORMS') == 'cpu' and 'jax' not in sys.modules:
    del os.environ['JAX_PLATFORMS']

import math
import numpy as np

T_MAX, BATCH, D, C = 512, 128, 256, 7
NCORE = 8
NCONV = BATCH // NCORE          # conversations per core

_BUILD_CACHE = {}


def _f8(x):
    """Host fp32 -> e4m3 bytes (clipped to the TRN-compatible +-240 range)."""
    import ml_dtypes
    return np.ascontiguousarray(
        np.clip(np.asarray(x, np.float32), -240.0, 240.0)
        .astype(ml_dtypes.float8_e4m3fn).view(np.uint8))


def _pow2_scale(w):
    s = float(np.std(w))
    if s == 0.0 or not np.isfinite(s):
        return 1.0
    return float(2.0 ** round(math.log2(4.0 / s)))


def _build(with_bias, slot_lens, scales):
    """Build + compile the SPMD Bass program. Returns the Bacc instance."""
    from contextlib import ExitStack
    import concourse.bacc as bacc
    import concourse.bass as bass  # noqa: F401
    from concourse import mybir, tile

    f32 = mybir.dt.float32
    bf16 = mybir.dt.bfloat16
    fp8 = mybir.dt.float8e4
    u8 = mybir.dt.uint8
    AF = mybir.ActivationFunctionType
    ALU = mybir.AluOpType
    AX = mybir.AxisListType
    PM = mybir.MatmulPerfMode.DoubleRow

    ws_e = {0: scales['ws_e_s'], 1: scales['ws_e_p']}
    ws_h = {0: scales['ws_h_s'], 1: scales['ws_h_p']}
    ows = scales['ows']

    nc = bacc.Bacc("TRN2", target_bir_lowering=False, debug=False,
                   num_devices=NCORE)

    def din(name, shape, dt):
        return nc.dram_tensor(name, shape, dt, kind="ExternalInput").ap()

    xt_d = {0: din("xts", [NCONV, 2, 128, T_MAX], u8),
            1: din("xtp", [NCONV, 2, 128, T_MAX], u8)}
    xn_d = {0: din("xns", [T_MAX, NCONV, D], u8),
            1: din("xnp", [T_MAX, NCONV, D], u8)}
    wdefs = {}
    for sti, st in enumerate(("s", "p")):
        wdefs[sti] = dict(
            we=din(f"we_{st}", [D, 768], u8),
            wh=din(f"wh_{st}", [D, 1024], u8),
            wr=din(f"wr_{st}", [D, 1024], u8),
            b1=din(f"b1_{st}", [128, 6], f32) if with_bias else None,
            b2=din(f"b2_{st}", [128, 8], f32) if with_bias else None,
        )
    m8_d = din("m8", [NCONV, 128, 512], u8)   # 0/1 row-validity, per ut block
    outw_d = din("outw", [4 * D, C], u8)
    out_d = nc.dram_tensor("out", [NCONV, T_MAX, C], f32,
                           kind="ExternalOutput").ap()

    with ExitStack() as ctx:
        tc = ctx.enter_context(tile.TileContext(nc))
        const = ctx.enter_context(tc.tile_pool(name="const", bufs=1))
        xpool = ctx.enter_context(tc.tile_pool(name="xpool", bufs=10))
        work = ctx.enter_context(tc.tile_pool(name="work", bufs=2))
        fpool = ctx.enter_context(tc.tile_pool(name="fpool", bufs=3))
        opool = ctx.enter_context(tc.tile_pool(name="opool", bufs=2))
        gp = ctx.enter_context(tc.tile_pool(name="gp", bufs=2, space="PSUM"))
        ep = ctx.enter_context(tc.tile_pool(name="ep", bufs=2, space="PSUM"))

        # ---- constants / weights (tiles now; DMAs ordered by first use) --
        W = {}
        for sti, st in enumerate(("s", "p")):
            d = wdefs[sti]
            we_t = const.tile([128, 2, 768], fp8, name=f"we_t{st}")
            wh_t = const.tile([128, 2, 1024], fp8, name=f"wh_t{st}")
            wr_t = const.tile([128, 2, 1024], fp8, name=f"wr_t{st}")
            b1_t = b2_t = None
            if with_bias:
                b1_t = const.tile([128, 6], f32, name=f"b1_t{st}")
                b2_t = const.tile([128, 8], f32, name=f"b2_t{st}")
            W[sti] = dict(we=we_t, wh=wh_t, wr=wr_t, b1=b1_t, b2=b2_t)
        m8_t = const.tile([128, NCONV, 4, 128], fp8, name="m8_t")
        outw_t = const.tile([128, 8, C], fp8, name="outw_t")
        lg_all = const.tile([128, NCONV, 4, C], f32, name="lg_all")
        s_all = const.tile([128, NCONV * 4], f32, name="s_all")
        lnS = const.tile([128, NCONV * 4], f32, name="lnS")

        def emit_const_dma(phase):
            for sti, st in enumerate(("s", "p")):
                d, w = wdefs[sti], W[sti]
                if phase == 0:      # needed by F(0) immediately
                    nc.sync.dma_start(out=w["we"], in_=d["we"].bitcast(fp8)
                                      .rearrange("(kt p) m -> p kt m", p=128))
                    if with_bias:
                        nc.sync.dma_start(out=w["b1"], in_=d["b1"])
                        nc.sync.dma_start(out=w["b2"], in_=d["b2"])
                else:               # needed from B1(0)/B2(0) onward
                    nc.sync.dma_start(out=w["wh"], in_=d["wh"].bitcast(fp8)
                                      .rearrange("(kt p) m -> p kt m", p=128))
                    nc.sync.dma_start(out=w["wr"], in_=d["wr"].bitcast(fp8)
                                      .rearrange("(kt p) m -> p kt m", p=128))
            if phase == 1:
                nc.sync.dma_start(out=m8_t, in_=m8_d.bitcast(fp8).rearrange(
                    "j p (b c) -> p j b c", b=4))
                nc.sync.dma_start(out=outw_t, in_=outw_d.bitcast(fp8).rearrange(
                    "(kt p) c -> p kt c", p=128))

        def mm(ps, lhsT, rhs, start, stop, pm=PM):
            nc.tensor.matmul(ps, lhsT, rhs, start=start, stop=stop,
                             perf_mode=pm)

        def dims(j):
            Lv = int(slot_lens[j])
            # 16-aligned: DoubleRow LDWEIGHTS requires k-pair step % 16 == 0
            L = min(T_MAX, ((Lv + 15) // 16) * 16)
            UT = (Lv + 127) // 128
            return Lv, L, UT

        S = [None] * NCONV      # per-conv pipeline state

        def gate_act(pg_ap, out_ap, nsl, scale, bias_t, bcol):
            """[128,nsl,L] psum -> bf16; merged unless per-z biases needed."""
            if with_bias:
                for z in range(nsl):
                    nc.scalar.activation(out_ap[:, z, :], pg_ap[:, z, :],
                                         AF.Tanh, scale=scale,
                                         bias=bias_t[:, bcol + z:bcol + z + 1])
            else:
                nc.scalar.activation(out_ap, pg_ap, AF.Tanh, scale=scale)

        def emit_dma(j):
            Lv, L, UT = dims(j)
            LX = UT * 128
            xt_, xn_ = {}, {}
            for st in (0, 1):
                xt = xpool.tile([128, 2, LX], fp8, tag="xt", name=f"xt{j}_{st}")
                for kd in range(2):
                    nc.sync.dma_start(out=xt[:, kd, :],
                                      in_=xt_d[st].bitcast(fp8)[j, kd, :, 0:LX])
                xn = xpool.tile([128, 4, D], fp8, tag="xn", name=f"xn{j}_{st}")
                for ut in range(UT):
                    nc.sync.dma_start(
                        out=xn[:, ut, :],
                        in_=xn_d[st].bitcast(fp8)[ut * 128:(ut + 1) * 128, j, :])
                xt_[st], xn_[st] = xt, xn
            S[j] = dict(xt=xt_, xn=xn_)

        def gen_F(j):
            Lv, L, UT = dims(j)
            st_ = S[j]
            g1_ = {}
            for st in (0, 1):
                w = W[st]
                gts = []
                for gi in range(3):            # (i0,i1) (g0,g1) (o0,o1)
                    pg_t = gp.tile([128, 2, T_MAX], f32, tag="pg",
                                   name=f"pg1{j}_{st}_{gi}")
                    for z in range(2):
                        m = 2 * gi + z
                        mm(pg_t[:, z, 0:L], w["we"][:, :, m * 128:(m + 1) * 128],
                           st_['xt'][st][:, :, 0:L], True, True)
                    gt = work.tile([128, 2, L], bf16, tag="g1", bufs=10,
                                   name=f"g1{j}_{st}_{gi}")
                    gate_act(pg_t[:, :, 0:L], gt[:, :, :], 2, 0.5 / ws_e[st],
                             w["b1"], 2 * gi)
                    gts.append(gt)
                g1_[st] = gts
                yield
            cs1 = work.tile([128, 4, L], bf16, tag="cs", bufs=6, name=f"cs1{j}")
            for st in (0, 1):
                nc.vector.scalar_tensor_tensor(cs1[:, 2 * st:2 * st + 2, :],
                                               g1_[st][0][:, :, :], 1.0,
                                               g1_[st][1][:, :, :],
                                               ALU.add, ALU.mult)
            th1 = work.tile([128, 4, L], bf16, tag="th", bufs=3, name=f"th1{j}")
            nc.scalar.activation(th1[:, :, :], cs1[:, :, :], AF.Tanh, scale=0.5)
            hs1_ = {}
            for st in (0, 1):
                hs1 = work.tile([128, 2, L], fp8, tag="hs", bufs=12,
                                name=f"hs1{j}_{st}")
                nc.vector.scalar_tensor_tensor(hs1[:, :, :],
                                               g1_[st][2][:, :, :], 1.0,
                                               th1[:, 2 * st:2 * st + 2, :],
                                               ALU.add, ALU.mult)
                hs1_[st] = hs1
            st_.update(cs1=cs1, hs1=hs1_)

        def attention(j, st, hs_tile, step, L, UT):
            """A = exp(0.5*e); Z = 1/(m8 . A) — masking via m8/zeroed-xn."""
            xt = S[j]['xt'][st]
            A = work.tile([128, 4, L], fp8, tag="A", bufs=4,
                          name=f"A{j}_{st}_{step}")
            done = 0
            while done < UT:
                take = 2 if UT - done >= 2 else 1
                et = ep.tile([128, 2, T_MAX], f32, tag="e",
                             name=f"e{j}_{st}_{step}_{done}")
                for q in range(take):
                    ut = done + q
                    mm(et[:, q, 0:L], xt[:, :, ut * 128:(ut + 1) * 128],
                       hs_tile[:, :, :], True, True)
                nc.scalar.activation(A[:, done:done + take, :],
                                     et[:, 0:take, 0:L], AF.Exp, scale=0.5)
                done += take
            NPAIR, ODD = UT // 2, UT % 2
            pt = ep.tile([128, 2, T_MAX], f32, tag="e", name=f"ps{j}_{st}_{step}")
            for k in range(NPAIR):
                mm(pt[:, 0, 0:L], m8_t[:, j, 2 * k:2 * k + 2, :],
                   A[:, 2 * k:2 * k + 2, :], k == 0,
                   k == NPAIR - 1 and not ODD)
            if ODD:
                mm(pt[:, 0, 0:L], m8_t[:, j, UT - 1, :], A[:, UT - 1, :],
                   NPAIR == 0, True, pm=None)
            Z = work.tile([128, L], f32, tag="Z", bufs=4, name=f"Z{j}_{st}_{step}")
            nc.vector.reciprocal_approx_fast(Z[:, :], pt[:, 0, 0:L])
            return A, Z

        def r_apply(j, st, A, Z, out_tile, zoff, relu, step, L, UT):
            """out[:, zoff+dt, :] = (X^T A) * Z, optionally relu'd."""
            xn = S[j]['xn'][st]
            NPAIR, ODD = UT // 2, UT % 2
            for dt in range(2):
                rt = ep.tile([128, 2, T_MAX], f32, tag="e",
                             name=f"r{j}_{st}_{step}_{dt}")
                for k in range(NPAIR):
                    mm(rt[:, 0, 0:L],
                       xn[:, 2 * k:2 * k + 2, dt * 128:(dt + 1) * 128],
                       A[:, 2 * k:2 * k + 2, :], k == 0,
                       k == NPAIR - 1 and not ODD)
                if ODD:
                    mm(rt[:, 0, 0:L], xn[:, UT - 1, dt * 128:(dt + 1) * 128],
                       A[:, UT - 1, :], NPAIR == 0, True, pm=None)
                nc.vector.scalar_tensor_tensor(
                    out_tile[:, zoff + dt, :], rt[:, 0, 0:L],
                    0.0 if relu else 1.0, Z[:, :],
                    ALU.max if relu else ALU.mult, ALU.mult)

        def gen_B1(j):
            Lv, L, UT = dims(j)
            st_ = S[j]
            AZ1, r1_ = {}, {}
            for st in (0, 1):
                AZ1[st] = attention(j, st, st_['hs1'][st], 1, L, UT)
                yield
            for st in (0, 1):
                r1 = work.tile([128, 2, L], fp8, tag="r1", bufs=8,
                               name=f"r1{j}_{st}")
                r_apply(j, st, AZ1[st][0], AZ1[st][1], r1, 0, False, 1, L, UT)
                r1_[st] = r1
                yield
            st_.update(r1=r1_)

        def gen_B2(j):
            Lv, L, UT = dims(j)
            st_ = S[j]
            g2_ = {}
            for st in (0, 1):
                w = W[st]
                gts = []
                for gi in range(4):            # i, f, g, o pairs
                    pg_t = gp.tile([128, 2, T_MAX], f32, tag="pg",
                                   name=f"pg2{j}_{st}_{gi}")
                    for z in range(2):
                        m = 2 * gi + z
                        mm(pg_t[:, z, 0:L],
                           w["wh"][:, :, m * 128:(m + 1) * 128],
                           st_['hs1'][st][:, :, :], True, False)
                        mm(pg_t[:, z, 0:L],
                           w["wr"][:, :, m * 128:(m + 1) * 128],
                           st_['r1'][st][:, :, :], False, True)
                    gt = work.tile([128, 2, L], bf16, tag="g2", bufs=10,
                                   name=f"g2{j}_{st}_{gi}")
                    gate_act(pg_t[:, :, 0:L], gt[:, :, :], 2, 0.5 / ws_h[st],
                             w["b2"], 2 * gi)
                    gts.append(gt)
                g2_[st] = gts                  # [i, f, g, o]
                yield
            cs2 = work.tile([128, 4, L], bf16, tag="cs", bufs=6, name=f"cs2{j}")
            for st in (0, 1):
                gi2, gf2, gg2, go2 = g2_[st]
                t1 = work.tile([128, 2, L], bf16, tag="tmp", bufs=4,
                               name=f"t1{j}_{st}")
                nc.vector.scalar_tensor_tensor(t1[:, :, :], gf2[:, :, :], 1.0,
                                               st_['cs1'][:, 2 * st:2 * st + 2, :],
                                               ALU.add, ALU.mult)
                t2 = work.tile([128, 2, L], bf16, tag="tmp", bufs=4,
                               name=f"t2{j}_{st}")
                nc.vector.scalar_tensor_tensor(t2[:, :, :], gi2[:, :, :], 1.0,
                                               gg2[:, :, :], ALU.add, ALU.mult)
                nc.vector.scalar_tensor_tensor(cs2[:, 2 * st:2 * st + 2, :],
                                               t1[:, :, :], 0.5, t2[:, :, :],
                                               ALU.mult, ALU.add)
            th2 = work.tile([128, 4, L], bf16, tag="th", bufs=3, name=f"th2{j}")
            nc.scalar.activation(th2[:, :, :], cs2[:, :, :], AF.Tanh, scale=0.5)
            hs2_ = {}
            for st in (0, 1):
                hs2 = work.tile([128, 2, L], fp8, tag="hs", bufs=12,
                                name=f"hs2{j}_{st}")
                nc.vector.scalar_tensor_tensor(hs2[:, :, :],
                                               g2_[st][3][:, :, :], 1.0,
                                               th2[:, 2 * st:2 * st + 2, :],
                                               ALU.add, ALU.mult)
                hs2_[st] = hs2
            st_.update(hs2=hs2_)

        def gen_B3(j):
            Lv, L, UT = dims(j)
            st_ = S[j]
            ft_ = {}
            for st in (0, 1):
                A2, Z2 = attention(j, st, st_['hs2'][st], 2, L, UT)
                yield
                ft = fpool.tile([128, 4, L], fp8, tag=f"ft{st}", name=f"ft{j}_{st}")
                nc.vector.tensor_scalar_max(ft[:, 0:2, :],
                                            st_['hs2'][st][:, :, :], 0.0)
                r_apply(j, st, A2, Z2, ft, 2, True, 2, L, UT)
                ft_[st] = ft
                yield
            # logits transposed: [t, C] per 128-t chunk (8-wide slots in psum)
            lpt = ep.tile([128, 2, T_MAX], f32, tag="e", name=f"lp{j}")
            for cch in range(UT):
                ncch = min(128, L - cch * 128)
                for m, (ftile, zz) in enumerate(
                        ((ft_[0], 0), (ft_[0], 2), (ft_[1], 0), (ft_[1], 2))):
                    mm(lpt[0:ncch, 0, cch * 8:cch * 8 + C],
                       ftile[:, zz:zz + 2, cch * 128:cch * 128 + ncch],
                       outw_t[:, 2 * m:2 * m + 2, :], m == 0, m == 3)
            lgv = lpt[:, 0, 0:UT * 8].rearrange("p (u c) -> p u c", c=8)[:, :, 0:C]
            nc.vector.tensor_scalar_mul(lg_all[:, j, 0:UT, :], lgv, 1.0 / ows)
            elg = work.tile([128, 4, C], f32, tag="elg", name=f"elg{j}")
            nc.scalar.activation(elg[:, 0:UT, :], lgv, AF.Exp, scale=1.0 / ows)
            nc.vector.tensor_reduce(s_all[:, j * 4:j * 4 + UT],
                                    elg[:, 0:UT, :], AX.X, ALU.add)
            S[j] = None

        # ---- 4-stage software-pipelined main loop ----------------------
        emit_const_dma(0)
        emit_dma(0)
        emit_dma(1)
        emit_const_dma(1)
        for t in range(NCONV + 3):
            if t + 2 < NCONV:
                emit_dma(t + 2)
            gens = []
            if t < NCONV:
                gens.append(gen_F(t))
            if t >= 1 and t - 1 < NCONV:
                gens.append(gen_B1(t - 1))
            if t >= 2 and t - 2 < NCONV:
                gens.append(gen_B2(t - 2))
            if t >= 3:
                gens.append(gen_B3(t - 3))
            while gens:
                nxt = []
                for g in gens:
                    try:
                        next(g)
                        nxt.append(g)
                    except StopIteration:
                        pass
                gens = nxt

        # ---- final: logp = lg - ln(rowsum) ----------------------------
        nc.scalar.activation(lnS[:, :], s_all[:, :], AF.Ln)
        for j in range(NCONV):
            UT = dims(j)[2]
            ot = opool.tile([128, 4, C], f32, tag="ot", name=f"ot{j}")
            for cch in range(UT):
                nc.gpsimd.tensor_scalar_sub(ot[:, cch, :], lg_all[:, j, cch, :],
                                            lnS[:, j * 4 + cch:j * 4 + cch + 1])
            nc.sync.dma_start(
                out=out_d[j].rearrange("(c p) k -> p c k", p=128)[:, 0:UT, :],
                in_=ot[:, 0:UT, :])

    nc.compile()
    return nc


def _host_prep(inputs):
    """Fold weights, quantize to fp8, pick conversation->core assignment."""
    x_s = np.asarray(inputs["input"], dtype=np.float32)
    x_p = np.asarray(inputs["speakers"], dtype=np.float32)
    lengths = np.asarray(inputs["utterance_lengths"]).astype(np.int64)
    fc_w = np.asarray(inputs["fc_w"], dtype=np.float32)
    fc_b = np.asarray(inputs["fc_b"], dtype=np.float32)
    out_w = np.asarray(inputs["out_w"], dtype=np.float32)
    out_b = np.asarray(inputs["out_b"], dtype=np.float32)

    per_stream = {}
    scales = {}
    any_b = False
    for st in ("s", "p"):
        w_ih = np.asarray(inputs[f"w_ih_{st}"], dtype=np.float32)
        w_hh = np.asarray(inputs[f"w_hh_{st}"], dtype=np.float32)
        b_ih = np.asarray(inputs[f"b_ih_{st}"], dtype=np.float32)
        b_hh = np.asarray(inputs[f"b_hh_{st}"], dtype=np.float32)
        W_eff = w_ih @ fc_w                          # [1024, 256]
        bias1 = w_ih @ fc_b + b_ih + b_hh            # [1024]
        sel = np.r_[0:D, 2 * D:4 * D]                # i, g, o rows
        We = np.ascontiguousarray(W_eff[sel].T)      # [256, 768]
        We[:, D:2 * D] *= 2.0                        # g-gate doubling
        Wh = np.ascontiguousarray((0.5 * (w_ih[:, :D] + w_hh)).T)  # [256, 1024]
        Wr = np.ascontiguousarray(w_ih[:, D:].T)     # [256, 1024]
        Wh[:, 2 * D:3 * D] *= 2.0
        Wr[:, 2 * D:3 * D] *= 2.0
        ws_e = _pow2_scale(We)
        ws_h = _pow2_scale(np.concatenate([Wh, Wr], axis=0))
        scales[f'ws_e_{st}'] = ws_e
        scales[f'ws_h_{st}'] = ws_h
        # per-slice activation biases (pre-multiplied by the tanh input
        # scale: 0.5 normally, 1.0 for the doubled g-gate)
        b1_sel = bias1[sel]                          # [768] i,g,o
        bias2 = b_ih + b_hh                          # [1024] i,f,g,o
        b1_cols = np.zeros((128, 6), np.float32)
        for m in range(6):
            f = 1.0 if m in (2, 3) else 0.5
            b1_cols[:, m] = f * b1_sel[m * 128:(m + 1) * 128]
        b2_cols = np.zeros((128, 8), np.float32)
        for m in range(8):
            f = 1.0 if m in (4, 5) else 0.5
            b2_cols[:, m] = f * bias2[m * 128:(m + 1) * 128]
        any_b |= bool(np.any(b1_cols != 0.0) or np.any(b2_cols != 0.0))
        per_stream[st] = (_f8(We * ws_e), _f8(Wh * ws_h), _f8(Wr * ws_h),
                          b1_cols, b2_cols)

    # out_w columns for the h-halves get the 0.5 compensation (h stored as 2h)
    ow = out_w.copy()
    ow[:, 0:D] *= 0.5
    ow[:, 2 * D:3 * D] *= 0.5
    ows = _pow2_scale(ow)
    scales['ows'] = ows
    outw8 = _f8(ow.T * ows)                          # [1024, 7]
    host_out_b = out_b

    # conversation -> (core, slot): sort by length desc, round-robin
    order = np.argsort(-lengths, kind="stable")
    assign = {}
    for rank, conv in enumerate(order):
        assign[int(conv)] = (rank % NCORE, rank // NCORE)
    order_lens = lengths[order]
    slot_lens = tuple(int(order_lens[8 * k]) for k in range(NCONV))

    # fp8-quantize the banks once (identical bytes for both layouts)
    import ml_dtypes
    xs8 = np.clip(x_s, -240.0, 240.0).astype(ml_dtypes.float8_e4m3fn).view(np.uint8)
    xp8 = np.clip(x_p, -240.0, 240.0).astype(ml_dtypes.float8_e4m3fn).view(np.uint8)
    one8 = int(np.array([1.0], dtype=ml_dtypes.float8_e4m3fn).view(np.uint8)[0])

    in_maps = []
    core_convs = []
    for core in range(NCORE):
        ids = [None] * NCONV
        for conv, (c, s) in assign.items():
            if c == core:
                ids[s] = conv
        core_convs.append(ids)
        m8 = np.zeros((NCONV, 128, 512), dtype=np.uint8)
        xns = xs8[:, ids, :].copy()      # [T_MAX, NCONV, D], u-major
        xnp = xp8[:, ids, :].copy()
        for s, conv in enumerate(ids):
            Lc = int(lengths[conv])
            valid = (np.arange(T_MAX) < Lc)
            m8[s, :, :] = np.where(valid, one8, 0).astype(np.uint8).reshape(
                4, 128).T.repeat(128, axis=1).reshape(128, 512)
            xns[Lc:, s, :] = 0
            xnp[Lc:, s, :] = 0
        im = {
            "xts": np.ascontiguousarray(
                xs8[:, ids, :].transpose(1, 2, 0).reshape(NCONV, 2, 128, T_MAX)),
            "xtp": np.ascontiguousarray(
                xp8[:, ids, :].transpose(1, 2, 0).reshape(NCONV, 2, 128, T_MAX)),
            "xns": np.ascontiguousarray(xns),
            "xnp": np.ascontiguousarray(xnp),
            "m8": m8,
            "outw": outw8,
        }
        for st in ("s", "p"):
            We8, Wh8, Wr8, b1c, b2c = per_stream[st]
            im[f"we_{st}"] = We8
            im[f"wh_{st}"] = Wh8
            im[f"wr_{st}"] = Wr8
            if any_b:
                im[f"b1_{st}"] = b1c
                im[f"b2_{st}"] = b2c
        in_maps.append(im)
    key = (any_b, slot_lens,
           tuple(sorted((k, float(v)) for k, v in scales.items())))
    return in_maps, core_convs, lengths, key, scales, host_out_b


def _gather(results, core_convs, lengths, out_b):
    """results: per-core {'out': [NCONV, T_MAX, C]} -> [sum(len), C]."""
    where = {}
    for core, ids in enumerate(core_convs):
        for slot, conv in enumerate(ids):
            where[conv] = (core, slot)
    chunks = []
    nz = bool(np.any(out_b != 0.0))
    for b in range(BATCH):
        core, slot = where[b]
        L = int(lengths[b])
        lg = results[core]["out"][slot, :L, :]
        if nz:
            # device log-softmax omitted out_b; log_softmax is shift-invariant
            # per row, so redo it with the bias added.
            lg = lg + out_b[None, :]
            lg = lg - np.log(np.exp(lg).sum(axis=1, keepdims=True))
        chunks.append(np.ascontiguousarray(lg))
    return np.concatenate(chunks, axis=0).astype(np.float32)


def _get_nc(key, scales):
    if key not in _BUILD_CACHE:
        _BUILD_CACHE[key] = _build(key[0], key[1], scales)
    return _BUILD_CACHE[key]


def kernel(**inputs):
    from concourse import bass_utils
    in_maps, core_convs, lengths, key, scales, out_b = _host_prep(inputs)
    nc = _get_nc(key, scales)
    res = bass_utils.run_bass_kernel_spmd(nc, in_maps, core_ids=list(range(NCORE)))
    return _gather(res.results, core_convs, lengths, out_b)


# revision 29
# speedup vs baseline: 1.0208x; 1.0208x over previous
"""DCRNCognition Trainium2 kernel — fp8 DoubleRow edition.

Self-contained: builds a Bass/Tile SPMD program for 8 NeuronCores, shards the
batch (conversation) axis across cores, runs via run_bass_kernel_spmd, and
gathers the valid positions on the host.

Math restructuring (identical to the verified baseline, rel err ~9e-7 in f32):
  - fc layer folded into step-1 LSTM gates; step-1 f-gate/c-init dead
  - step-2: gates2 = hs1 @ Wh.T + r1 @ Wr.T  (Wh = 0.5*(w_ih[:, :D]+w_hh))
  - softmax normalization deferred to r:  r = (X^T A) * (1/sum_u A)
  - sigmoid via tanh; h,c carried scaled by 2 (hs=2h, cs=2c)

Precision plan (validated on host: rel err ~6e-4 vs the 2e-2 gate):
  - ALL matmuls fp8 e4m3 with DoubleRow perf mode, fp32 PSUM accum.
  - weights pre-scaled by a power of 2 into fp8 range on the host; the
    compensation folds into the free activation `scale` operands.
  - g-gate weight rows doubled on host so every gate activation shares
    scale=0.5 -> one ACT instruction per gate pair.
  - masking without exp bias: invalid bank rows (u >= len) are zeroed on
    the host in the u-major layout (kills their r contribution) and the
    softmax denominator contracts A against a per-conversation 0/1 mask
    as the matmul stationary (kills them in the sum).
  - logits computed transposed ([t,C]): log-softmax along the free axis,
    one batched Ln at the end -> 2 ACT table loads total.

Scheduling: 4-stage software pipeline (F=gates1+cell1, B1=attention1,
B2=gates2+cell2, B3=attention2+logits) with a 3-conversation skew so every
engine queue always holds ready work. PSUM: two 4-bank pools (gates /
attention+logits).
"""
import os
import sys
sys.path.insert(0, '/opt/trn_rl_repo')

# run_bass_kernel_spmd executes through jax/PJRT on the axon-tunneled
# NeuronCores; a JAX_PLATFORMS=cpu pin would hide them.
if os.environ.get('JAX_PLATFORMS') == 'cpu' and 'jax' not in sys.modules:
    del os.environ['JAX_PLATFORMS']

import math
import numpy as np

T_MAX, BATCH, D, C = 512, 128, 256, 7
NCORE = 8
NCONV = BATCH // NCORE          # conversations per core

_BUILD_CACHE = {}


def _f8(x):
    """Host fp32 -> e4m3 bytes (clipped to the TRN-compatible +-240 range)."""
    import ml_dtypes
    return np.ascontiguousarray(
        np.clip(np.asarray(x, np.float32), -240.0, 240.0)
        .astype(ml_dtypes.float8_e4m3fn).view(np.uint8))


def _pow2_scale(w):
    s = float(np.std(w))
    if s == 0.0 or not np.isfinite(s):
        return 1.0
    return float(2.0 ** round(math.log2(4.0 / s)))


def _build(with_bias, slot_lens, scales):
    """Build + compile the SPMD Bass program. Returns the Bacc instance."""
    from contextlib import ExitStack
    import concourse.bacc as bacc
    import concourse.bass as bass  # noqa: F401
    from concourse import mybir, tile

    f32 = mybir.dt.float32
    bf16 = mybir.dt.bfloat16
    fp8 = mybir.dt.float8e4
    u8 = mybir.dt.uint8
    AF = mybir.ActivationFunctionType
    ALU = mybir.AluOpType
    AX = mybir.AxisListType
    PM = mybir.MatmulPerfMode.DoubleRow

    ws_e = {0: scales['ws_e_s'], 1: scales['ws_e_p']}
    ws_h = {0: scales['ws_h_s'], 1: scales['ws_h_p']}
    ows = scales['ows']

    nc = bacc.Bacc("TRN2", target_bir_lowering=False, debug=False,
                   num_devices=NCORE)

    def din(name, shape, dt):
        return nc.dram_tensor(name, shape, dt, kind="ExternalInput").ap()

    xt_d = {0: din("xts", [NCONV, 2, 128, T_MAX], u8),
            1: din("xtp", [NCONV, 2, 128, T_MAX], u8)}
    xn_d = {0: din("xns", [T_MAX, NCONV, D], u8),
            1: din("xnp", [T_MAX, NCONV, D], u8)}
    wdefs = {}
    for sti, st in enumerate(("s", "p")):
        wdefs[sti] = dict(
            we=din(f"we_{st}", [D, 768], u8),
            wh=din(f"wh_{st}", [D, 1024], u8),
            wr=din(f"wr_{st}", [D, 1024], u8),
            b1=din(f"b1_{st}", [128, 6], f32) if with_bias else None,
            b2=din(f"b2_{st}", [128, 8], f32) if with_bias else None,
        )
    m8_d = din("m8", [NCONV, 128, 512], u8)   # 0/1 row-validity, per ut block
    outw_d = din("outw", [4 * D, C], u8)
    out_d = nc.dram_tensor("out", [NCONV, T_MAX, C], f32,
                           kind="ExternalOutput").ap()

    with ExitStack() as ctx:
        tc = ctx.enter_context(tile.TileContext(nc))
        const = ctx.enter_context(tc.tile_pool(name="const", bufs=1))
        xpool = ctx.enter_context(tc.tile_pool(name="xpool", bufs=10))
        work = ctx.enter_context(tc.tile_pool(name="work", bufs=2))
        fpool = ctx.enter_context(tc.tile_pool(name="fpool", bufs=3))
        opool = ctx.enter_context(tc.tile_pool(name="opool", bufs=2))
        gp = ctx.enter_context(tc.tile_pool(name="gp", bufs=2, space="PSUM"))
        ep = ctx.enter_context(tc.tile_pool(name="ep", bufs=2, space="PSUM"))

        # ---- constants / weights (tiles now; DMAs ordered by first use) --
        W = {}
        for sti, st in enumerate(("s", "p")):
            d = wdefs[sti]
            we_t = const.tile([128, 2, 768], fp8, name=f"we_t{st}")
            wh_t = const.tile([128, 2, 1024], fp8, name=f"wh_t{st}")
            wr_t = const.tile([128, 2, 1024], fp8, name=f"wr_t{st}")
            b1_t = b2_t = None
            if with_bias:
                b1_t = const.tile([128, 6], f32, name=f"b1_t{st}")
                b2_t = const.tile([128, 8], f32, name=f"b2_t{st}")
            W[sti] = dict(we=we_t, wh=wh_t, wr=wr_t, b1=b1_t, b2=b2_t)
        m8_t = const.tile([128, NCONV, 4, 128], fp8, name="m8_t")
        outw_t = const.tile([128, 8, C], fp8, name="outw_t")
        lg_all = const.tile([128, NCONV, 4, C], f32, name="lg_all")
        s_all = const.tile([128, NCONV * 4], f32, name="s_all")
        lnS = const.tile([128, NCONV * 4], f32, name="lnS")

        def emit_const_dma(phase):
            for sti, st in enumerate(("s", "p")):
                d, w = wdefs[sti], W[sti]
                if phase == 0:      # needed by F(0) immediately
                    nc.sync.dma_start(out=w["we"], in_=d["we"].bitcast(fp8)
                                      .rearrange("(kt p) m -> p kt m", p=128))
                    if with_bias:
                        nc.sync.dma_start(out=w["b1"], in_=d["b1"])
                        nc.sync.dma_start(out=w["b2"], in_=d["b2"])
                else:               # needed from B1(0)/B2(0) onward
                    nc.sync.dma_start(out=w["wh"], in_=d["wh"].bitcast(fp8)
                                      .rearrange("(kt p) m -> p kt m", p=128))
                    nc.sync.dma_start(out=w["wr"], in_=d["wr"].bitcast(fp8)
                                      .rearrange("(kt p) m -> p kt m", p=128))
            if phase == 1:
                nc.sync.dma_start(out=m8_t, in_=m8_d.bitcast(fp8).rearrange(
                    "j p (b c) -> p j b c", b=4))
                nc.sync.dma_start(out=outw_t, in_=outw_d.bitcast(fp8).rearrange(
                    "(kt p) c -> p kt c", p=128))

        def mm(ps, lhsT, rhs, start, stop, pm=PM):
            nc.tensor.matmul(ps, lhsT, rhs, start=start, stop=stop,
                             perf_mode=pm)

        def dims(j):
            Lv = int(slot_lens[j])
            # 16-aligned: DoubleRow LDWEIGHTS requires k-pair step % 16 == 0
            L = min(T_MAX, ((Lv + 15) // 16) * 16)
            UT = (Lv + 127) // 128
            return Lv, L, UT

        S = [None] * NCONV      # per-conv pipeline state

        def gate_act(pg_ap, out_ap, nsl, scale, bias_t, bcol):
            """[128,nsl,L] psum -> bf16; merged unless per-z biases needed."""
            if with_bias:
                for z in range(nsl):
                    nc.scalar.activation(out_ap[:, z, :], pg_ap[:, z, :],
                                         AF.Tanh, scale=scale,
                                         bias=bias_t[:, bcol + z:bcol + z + 1])
            else:
                nc.scalar.activation(out_ap, pg_ap, AF.Tanh, scale=scale)

        def emit_dma(j):
            Lv, L, UT = dims(j)
            LX = UT * 128
            xt_, xn_ = {}, {}
            for st in (0, 1):
                xt = xpool.tile([128, 2, LX], fp8, tag="xt", name=f"xt{j}_{st}")
                for kd in range(2):
                    nc.sync.dma_start(out=xt[:, kd, :],
                                      in_=xt_d[st].bitcast(fp8)[j, kd, :, 0:LX])
                xn = xpool.tile([128, 4, D], fp8, tag="xn", name=f"xn{j}_{st}")
                for ut in range(UT):
                    nc.sync.dma_start(
                        out=xn[:, ut, :],
                        in_=xn_d[st].bitcast(fp8)[ut * 128:(ut + 1) * 128, j, :])
                xt_[st], xn_[st] = xt, xn
            S[j] = dict(xt=xt_, xn=xn_)

        def gen_F(j):
            Lv, L, UT = dims(j)
            st_ = S[j]
            g1_ = {}
            for st in (0, 1):
                w = W[st]
                gts = []
                for gi in range(3):            # (i0,i1) (g0,g1) (o0,o1)
                    pg_t = gp.tile([128, 2, T_MAX], f32, tag="pg",
                                   name=f"pg1{j}_{st}_{gi}")
                    for z in range(2):
                        m = 2 * gi + z
                        mm(pg_t[:, z, 0:L], w["we"][:, :, m * 128:(m + 1) * 128],
                           st_['xt'][st][:, :, 0:L], True, True)
                    gt = work.tile([128, 2, L], bf16, tag="g1", bufs=10,
                                   name=f"g1{j}_{st}_{gi}")
                    gate_act(pg_t[:, :, 0:L], gt[:, :, :], 2, 0.5 / ws_e[st],
                             w["b1"], 2 * gi)
                    gts.append(gt)
                g1_[st] = gts
                yield
            cs1 = work.tile([128, 4, L], bf16, tag="cs", bufs=6, name=f"cs1{j}")
            for st in (0, 1):
                nc.vector.scalar_tensor_tensor(cs1[:, 2 * st:2 * st + 2, :],
                                               g1_[st][0][:, :, :], 1.0,
                                               g1_[st][1][:, :, :],
                                               ALU.add, ALU.mult)
            th1 = work.tile([128, 4, L], bf16, tag="th", bufs=3, name=f"th1{j}")
            nc.scalar.activation(th1[:, :, :], cs1[:, :, :], AF.Tanh, scale=0.5)
            hs1_ = {}
            for st in (0, 1):
                hs1 = work.tile([128, 2, L], fp8, tag="hs", bufs=12,
                                name=f"hs1{j}_{st}")
                nc.vector.scalar_tensor_tensor(hs1[:, :, :],
                                               g1_[st][2][:, :, :], 1.0,
                                               th1[:, 2 * st:2 * st + 2, :],
                                               ALU.add, ALU.mult)
                hs1_[st] = hs1
            st_.update(cs1=cs1, hs1=hs1_)

        def attention(j, st, hs_tile, step, L, UT):
            """A = exp(0.5*e); Z = 1/(m8 . A) — masking via m8/zeroed-xn."""
            xt = S[j]['xt'][st]
            A = work.tile([128, 4, L], fp8, tag="A", bufs=4,
                          name=f"A{j}_{st}_{step}")
            done = 0
            while done < UT:
                take = 2 if UT - done >= 2 else 1
                et = ep.tile([128, 2, T_MAX], f32, tag="e",
                             name=f"e{j}_{st}_{step}_{done}")
                for q in range(take):
                    ut = done + q
                    mm(et[:, q, 0:L], xt[:, :, ut * 128:(ut + 1) * 128],
                       hs_tile[:, :, :], True, True)
                nc.scalar.activation(A[:, done:done + take, :],
                                     et[:, 0:take, 0:L], AF.Exp, scale=0.5)
                done += take
            NPAIR, ODD = UT // 2, UT % 2
            pt = ep.tile([128, 2, T_MAX], f32, tag="e", name=f"ps{j}_{st}_{step}")
            for k in range(NPAIR):
                mm(pt[:, 0, 0:L], m8_t[:, j, 2 * k:2 * k + 2, :],
                   A[:, 2 * k:2 * k + 2, :], k == 0,
                   k == NPAIR - 1 and not ODD)
            if ODD:
                mm(pt[:, 0, 0:L], m8_t[:, j, UT - 1, :], A[:, UT - 1, :],
                   NPAIR == 0, True, pm=None)
            Z = work.tile([128, L], f32, tag="Z", bufs=4, name=f"Z{j}_{st}_{step}")
            nc.vector.reciprocal_approx_fast(Z[:, :], pt[:, 0, 0:L])
            return A, Z

        def r_apply(j, st, A, Z, out_tile, zoff, relu, step, L, UT):
            """out[:, zoff+dt, :] = (X^T A) * Z, optionally relu'd."""
            xn = S[j]['xn'][st]
            NPAIR, ODD = UT // 2, UT % 2
            for dt in range(2):
                rt = ep.tile([128, 2, T_MAX], f32, tag="e",
                             name=f"r{j}_{st}_{step}_{dt}")
                for k in range(NPAIR):
                    mm(rt[:, 0, 0:L],
                       xn[:, 2 * k:2 * k + 2, dt * 128:(dt + 1) * 128],
                       A[:, 2 * k:2 * k + 2, :], k == 0,
                       k == NPAIR - 1 and not ODD)
                if ODD:
                    mm(rt[:, 0, 0:L], xn[:, UT - 1, dt * 128:(dt + 1) * 128],
                       A[:, UT - 1, :], NPAIR == 0, True, pm=None)
                nc.vector.scalar_tensor_tensor(
                    out_tile[:, zoff + dt, :], rt[:, 0, 0:L],
                    0.0 if relu else 1.0, Z[:, :],
                    ALU.max if relu else ALU.mult, ALU.mult)

        def gen_B1(j):
            Lv, L, UT = dims(j)
            st_ = S[j]
            AZ1, r1_ = {}, {}
            for st in (0, 1):
                AZ1[st] = attention(j, st, st_['hs1'][st], 1, L, UT)
                yield
            for st in (0, 1):
                r1 = work.tile([128, 2, L], fp8, tag="r1", bufs=8,
                               name=f"r1{j}_{st}")
                r_apply(j, st, AZ1[st][0], AZ1[st][1], r1, 0, False, 1, L, UT)
                r1_[st] = r1
                yield
            st_.update(r1=r1_)

        def gen_B2(j):
            Lv, L, UT = dims(j)
            st_ = S[j]
            g2_ = {}
            for st in (0, 1):
                w = W[st]
                gts = []
                for gi in range(4):            # i, f, g, o pairs
                    pg_t = gp.tile([128, 2, T_MAX], f32, tag="pg",
                                   name=f"pg2{j}_{st}_{gi}")
                    for z in range(2):
                        m = 2 * gi + z
                        mm(pg_t[:, z, 0:L],
                           w["wh"][:, :, m * 128:(m + 1) * 128],
                           st_['hs1'][st][:, :, :], True, False)
                        mm(pg_t[:, z, 0:L],
                           w["wr"][:, :, m * 128:(m + 1) * 128],
                           st_['r1'][st][:, :, :], False, True)
                    gt = work.tile([128, 2, L], bf16, tag="g2", bufs=10,
                                   name=f"g2{j}_{st}_{gi}")
                    gate_act(pg_t[:, :, 0:L], gt[:, :, :], 2, 0.5 / ws_h[st],
                             w["b2"], 2 * gi)
                    gts.append(gt)
                g2_[st] = gts                  # [i, f, g, o]
                yield
            cs2 = work.tile([128, 4, L], bf16, tag="cs", bufs=6, name=f"cs2{j}")
            for st in (0, 1):
                gi2, gf2, gg2, go2 = g2_[st]
                t1 = work.tile([128, 2, L], bf16, tag="tmp", bufs=4,
                               name=f"t1{j}_{st}")
                nc.vector.scalar_tensor_tensor(t1[:, :, :], gf2[:, :, :], 1.0,
                                               st_['cs1'][:, 2 * st:2 * st + 2, :],
                                               ALU.add, ALU.mult)
                t2 = work.tile([128, 2, L], bf16, tag="tmp", bufs=4,
                               name=f"t2{j}_{st}")
                nc.vector.scalar_tensor_tensor(t2[:, :, :], gi2[:, :, :], 1.0,
                                               gg2[:, :, :], ALU.add, ALU.mult)
                nc.vector.scalar_tensor_tensor(cs2[:, 2 * st:2 * st + 2, :],
                                               t1[:, :, :], 0.5, t2[:, :, :],
                                               ALU.mult, ALU.add)
            th2 = work.tile([128, 4, L], bf16, tag="th", bufs=3, name=f"th2{j}")
            nc.scalar.activation(th2[:, :, :], cs2[:, :, :], AF.Tanh, scale=0.5)
            hs2_ = {}
            for st in (0, 1):
                hs2 = work.tile([128, 2, L], fp8, tag="hs", bufs=12,
                                name=f"hs2{j}_{st}")
                nc.vector.scalar_tensor_tensor(hs2[:, :, :],
                                               g2_[st][3][:, :, :], 1.0,
                                               th2[:, 2 * st:2 * st + 2, :],
                                               ALU.add, ALU.mult)
                hs2_[st] = hs2
            st_.update(hs2=hs2_)

        def gen_B3(j):
            Lv, L, UT = dims(j)
            st_ = S[j]
            ft_ = {}
            for st in (0, 1):
                A2, Z2 = attention(j, st, st_['hs2'][st], 2, L, UT)
                yield
                ft = fpool.tile([128, 4, L], fp8, tag=f"ft{st}", name=f"ft{j}_{st}")
                nc.vector.tensor_scalar_max(ft[:, 0:2, :],
                                            st_['hs2'][st][:, :, :], 0.0)
                r_apply(j, st, A2, Z2, ft, 2, True, 2, L, UT)
                ft_[st] = ft
                yield
            # logits transposed: [t, C] per 128-t chunk (8-wide slots in psum)
            lpt = ep.tile([128, 2, T_MAX], f32, tag="e", name=f"lp{j}")
            for cch in range(UT):
                ncch = min(128, L - cch * 128)
                for m, (ftile, zz) in enumerate(
                        ((ft_[0], 0), (ft_[0], 2), (ft_[1], 0), (ft_[1], 2))):
                    mm(lpt[0:ncch, 0, cch * 8:cch * 8 + C],
                       ftile[:, zz:zz + 2, cch * 128:cch * 128 + ncch],
                       outw_t[:, 2 * m:2 * m + 2, :], m == 0, m == 3)
            lgv = lpt[:, 0, 0:UT * 8].rearrange("p (u c) -> p u c", c=8)[:, :, 0:C]
            nc.vector.tensor_scalar_mul(lg_all[:, j, 0:UT, :], lgv, 1.0 / ows)
            elg = work.tile([128, 4, C], f32, tag="elg", name=f"elg{j}")
            nc.scalar.activation(elg[:, 0:UT, :], lgv, AF.Exp, scale=1.0 / ows)
            nc.vector.tensor_reduce(s_all[:, j * 4:j * 4 + UT],
                                    elg[:, 0:UT, :], AX.X, ALU.add)
            S[j] = None

        # ---- 4-stage software-pipelined main loop ----------------------
        emit_const_dma(0)
        emit_dma(0)
        emit_dma(1)
        emit_const_dma(1)
        for t in range(NCONV + 3):
            if t + 2 < NCONV:
                emit_dma(t + 2)
            gens = []
            if t < NCONV:
                gens.append(gen_F(t))
            if t >= 1 and t - 1 < NCONV:
                gens.append(gen_B1(t - 1))
            if t >= 2 and t - 2 < NCONV:
                gens.append(gen_B2(t - 2))
            if t >= 3:
                gens.append(gen_B3(t - 3))
            while gens:
                nxt = []
                for g in gens:
                    try:
                        next(g)
                        nxt.append(g)
                    except StopIteration:
                        pass
                gens = nxt

        # ---- final: logp = lg - ln(rowsum) ----------------------------
        nc.scalar.activation(lnS[:, :], s_all[:, :], AF.Ln)
        for j in range(NCONV):
            UT = dims(j)[2]
            ot = opool.tile([128, 4, C], f32, tag="ot", name=f"ot{j}")
            for cch in range(UT):
                nc.vector.tensor_scalar_sub(ot[:, cch, :], lg_all[:, j, cch, :],
                                            lnS[:, j * 4 + cch:j * 4 + cch + 1])
            nc.sync.dma_start(
                out=out_d[j].rearrange("(c p) k -> p c k", p=128)[:, 0:UT, :],
                in_=ot[:, 0:UT, :])

    nc.compile()
    return nc


def _host_prep(inputs):
    """Fold weights, quantize to fp8, pick conversation->core assignment."""
    x_s = np.asarray(inputs["input"], dtype=np.float32)
    x_p = np.asarray(inputs["speakers"], dtype=np.float32)
    lengths = np.asarray(inputs["utterance_lengths"]).astype(np.int64)
    fc_w = np.asarray(inputs["fc_w"], dtype=np.float32)
    fc_b = np.asarray(inputs["fc_b"], dtype=np.float32)
    out_w = np.asarray(inputs["out_w"], dtype=np.float32)
    out_b = np.asarray(inputs["out_b"], dtype=np.float32)

    per_stream = {}
    scales = {}
    any_b = False
    for st in ("s", "p"):
        w_ih = np.asarray(inputs[f"w_ih_{st}"], dtype=np.float32)
        w_hh = np.asarray(inputs[f"w_hh_{st}"], dtype=np.float32)
        b_ih = np.asarray(inputs[f"b_ih_{st}"], dtype=np.float32)
        b_hh = np.asarray(inputs[f"b_hh_{st}"], dtype=np.float32)
        W_eff = w_ih @ fc_w                          # [1024, 256]
        bias1 = w_ih @ fc_b + b_ih + b_hh            # [1024]
        sel = np.r_[0:D, 2 * D:4 * D]                # i, g, o rows
        We = np.ascontiguousarray(W_eff[sel].T)      # [256, 768]
        We[:, D:2 * D] *= 2.0                        # g-gate doubling
        Wh = np.ascontiguousarray((0.5 * (w_ih[:, :D] + w_hh)).T)  # [256, 1024]
        Wr = np.ascontiguousarray(w_ih[:, D:].T)     # [256, 1024]
        Wh[:, 2 * D:3 * D] *= 2.0
        Wr[:, 2 * D:3 * D] *= 2.0
        ws_e = _pow2_scale(We)
        ws_h = _pow2_scale(np.concatenate([Wh, Wr], axis=0))
        scales[f'ws_e_{st}'] = ws_e
        scales[f'ws_h_{st}'] = ws_h
        # per-slice activation biases (pre-multiplied by the tanh input
        # scale: 0.5 normally, 1.0 for the doubled g-gate)
        b1_sel = bias1[sel]                          # [768] i,g,o
        bias2 = b_ih + b_hh                          # [1024] i,f,g,o
        b1_cols = np.zeros((128, 6), np.float32)
        for m in range(6):
            f = 1.0 if m in (2, 3) else 0.5
            b1_cols[:, m] = f * b1_sel[m * 128:(m + 1) * 128]
        b2_cols = np.zeros((128, 8), np.float32)
        for m in range(8):
            f = 1.0 if m in (4, 5) else 0.5
            b2_cols[:, m] = f * bias2[m * 128:(m + 1) * 128]
        any_b |= bool(np.any(b1_cols != 0.0) or np.any(b2_cols != 0.0))
        per_stream[st] = (_f8(We * ws_e), _f8(Wh * ws_h), _f8(Wr * ws_h),
                          b1_cols, b2_cols)

    # out_w columns for the h-halves get the 0.5 compensation (h stored as 2h)
    ow = out_w.copy()
    ow[:, 0:D] *= 0.5
    ow[:, 2 * D:3 * D] *= 0.5
    ows = _pow2_scale(ow)
    scales['ows'] = ows
    outw8 = _f8(ow.T * ows)                          # [1024, 7]
    host_out_b = out_b

    # conversation -> (core, slot): sort by length desc, round-robin
    order = np.argsort(-lengths, kind="stable")
    assign = {}
    for rank, conv in enumerate(order):
        assign[int(conv)] = (rank % NCORE, rank // NCORE)
    order_lens = lengths[order]
    slot_lens = tuple(int(order_lens[8 * k]) for k in range(NCONV))

    # fp8-quantize the banks once (identical bytes for both layouts)
    import ml_dtypes
    xs8 = np.clip(x_s, -240.0, 240.0).astype(ml_dtypes.float8_e4m3fn).view(np.uint8)
    xp8 = np.clip(x_p, -240.0, 240.0).astype(ml_dtypes.float8_e4m3fn).view(np.uint8)
    one8 = int(np.array([1.0], dtype=ml_dtypes.float8_e4m3fn).view(np.uint8)[0])

    in_maps = []
    core_convs = []
    for core in range(NCORE):
        ids = [None] * NCONV
        for conv, (c, s) in assign.items():
            if c == core:
                ids[s] = conv
        core_convs.append(ids)
        m8 = np.zeros((NCONV, 128, 512), dtype=np.uint8)
        xns = xs8[:, ids, :].copy()      # [T_MAX, NCONV, D], u-major
        xnp = xp8[:, ids, :].copy()
        for s, conv in enumerate(ids):
            Lc = int(lengths[conv])
            valid = (np.arange(T_MAX) < Lc)
            m8[s, :, :] = np.where(valid, one8, 0).astype(np.uint8).reshape(
                4, 128).T.repeat(128, axis=1).reshape(128, 512)
            xns[Lc:, s, :] = 0
            xnp[Lc:, s, :] = 0
        im = {
            "xts": np.ascontiguousarray(
                xs8[:, ids, :].transpose(1, 2, 0).reshape(NCONV, 2, 128, T_MAX)),
            "xtp": np.ascontiguousarray(
                xp8[:, ids, :].transpose(1, 2, 0).reshape(NCONV, 2, 128, T_MAX)),
            "xns": np.ascontiguousarray(xns),
            "xnp": np.ascontiguousarray(xnp),
            "m8": m8,
            "outw": outw8,
        }
        for st in ("s", "p"):
            We8, Wh8, Wr8, b1c, b2c = per_stream[st]
            im[f"we_{st}"] = We8
            im[f"wh_{st}"] = Wh8
            im[f"wr_{st}"] = Wr8
            if any_b:
                im[f"b1_{st}"] = b1c
                im[f"b2_{st}"] = b2c
        in_maps.append(im)
    key = (any_b, slot_lens,
           tuple(sorted((k, float(v)) for k, v in scales.items())))
    return in_maps, core_convs, lengths, key, scales, host_out_b


def _gather(results, core_convs, lengths, out_b):
    """results: per-core {'out': [NCONV, T_MAX, C]} -> [sum(len), C]."""
    where = {}
    for core, ids in enumerate(core_convs):
        for slot, conv in enumerate(ids):
            where[conv] = (core, slot)
    chunks = []
    nz = bool(np.any(out_b != 0.0))
    for b in range(BATCH):
        core, slot = where[b]
        L = int(lengths[b])
        lg = results[core]["out"][slot, :L, :]
        if nz:
            # device log-softmax omitted out_b; log_softmax is shift-invariant
            # per row, so redo it with the bias added.
            lg = lg + out_b[None, :]
            lg = lg - np.log(np.exp(lg).sum(axis=1, keepdims=True))
        chunks.append(np.ascontiguousarray(lg))
    return np.concatenate(chunks, axis=0).astype(np.float32)


def _get_nc(key, scales):
    if key not in _BUILD_CACHE:
        _BUILD_CACHE[key] = _build(key[0], key[1], scales)
    return _BUILD_CACHE[key]


def kernel(**inputs):
    from concourse import bass_utils
    in_maps, core_convs, lengths, key, scales, out_b = _host_prep(inputs)
    nc = _get_nc(key, scales)
    res = bass_utils.run_bass_kernel_spmd(nc, in_maps, core_ids=list(range(NCORE)))
    return _gather(res.results, core_convs, lengths, out_b)


# revision 30
# speedup vs baseline: 1.0231x; 1.0022x over previous
"""DCRNCognition Trainium2 kernel — fp8 DoubleRow edition.

Self-contained: builds a Bass/Tile SPMD program for 8 NeuronCores, shards the
batch (conversation) axis across cores, runs via run_bass_kernel_spmd, and
gathers the valid positions on the host.

Math restructuring (identical to the verified baseline, rel err ~9e-7 in f32):
  - fc layer folded into step-1 LSTM gates; step-1 f-gate/c-init dead
  - step-2: gates2 = hs1 @ Wh.T + r1 @ Wr.T  (Wh = 0.5*(w_ih[:, :D]+w_hh))
  - softmax normalization deferred to r:  r = (X^T A) * (1/sum_u A)
  - sigmoid via tanh; h,c carried scaled by 2 (hs=2h, cs=2c)

Precision plan (validated on host: rel err ~6e-4 vs the 2e-2 gate):
  - ALL matmuls fp8 e4m3 with DoubleRow perf mode, fp32 PSUM accum.
  - weights pre-scaled by a power of 2 into fp8 range on the host; the
    compensation folds into the free activation `scale` operands.
  - g-gate weight rows doubled on host so every gate activation shares
    scale=0.5 -> one ACT instruction per gate pair.
  - masking without exp bias: invalid bank rows (u >= len) are zeroed on
    the host in the u-major layout (kills their r contribution) and the
    softmax denominator contracts A against a per-conversation 0/1 mask
    as the matmul stationary (kills them in the sum).
  - logits computed transposed ([t,C]): log-softmax along the free axis,
    one batched Ln at the end -> 2 ACT table loads total.

Scheduling: 4-stage software pipeline (F=gates1+cell1, B1=attention1,
B2=gates2+cell2, B3=attention2+logits) with a 3-conversation skew so every
engine queue always holds ready work. PSUM: two 4-bank pools (gates /
attention+logits).
"""
import os
import sys
sys.path.insert(0, '/opt/trn_rl_repo')

# run_bass_kernel_spmd executes through jax/PJRT on the axon-tunneled
# NeuronCores; a JAX_PLATFORMS=cpu pin would hide them.
if os.environ.get('JAX_PLATFORMS') == 'cpu' and 'jax' not in sys.modules:
    del os.environ['JAX_PLATFORMS']

import math
import numpy as np

T_MAX, BATCH, D, C = 512, 128, 256, 7
NCORE = 8
NCONV = BATCH // NCORE          # conversations per core

_BUILD_CACHE = {}


def _f8(x):
    """Host fp32 -> e4m3 bytes (clipped to the TRN-compatible +-240 range)."""
    import ml_dtypes
    return np.ascontiguousarray(
        np.clip(np.asarray(x, np.float32), -240.0, 240.0)
        .astype(ml_dtypes.float8_e4m3fn).view(np.uint8))


def _pow2_scale(w):
    s = float(np.std(w))
    if s == 0.0 or not np.isfinite(s):
        return 1.0
    return float(2.0 ** round(math.log2(4.0 / s)))


def _build(with_bias, slot_lens, scales):
    """Build + compile the SPMD Bass program. Returns the Bacc instance."""
    from contextlib import ExitStack
    import concourse.bacc as bacc
    import concourse.bass as bass  # noqa: F401
    from concourse import mybir, tile

    f32 = mybir.dt.float32
    bf16 = mybir.dt.bfloat16
    fp8 = mybir.dt.float8e4
    u8 = mybir.dt.uint8
    AF = mybir.ActivationFunctionType
    ALU = mybir.AluOpType
    AX = mybir.AxisListType
    PM = mybir.MatmulPerfMode.DoubleRow

    ws_e = {0: scales['ws_e_s'], 1: scales['ws_e_p']}
    ws_h = {0: scales['ws_h_s'], 1: scales['ws_h_p']}
    ows = scales['ows']

    nc = bacc.Bacc("TRN2", target_bir_lowering=False, debug=False,
                   num_devices=NCORE)

    def din(name, shape, dt):
        return nc.dram_tensor(name, shape, dt, kind="ExternalInput").ap()

    xt_d = {0: din("xts", [NCONV, 2, 128, T_MAX], u8),
            1: din("xtp", [NCONV, 2, 128, T_MAX], u8)}
    xn_d = {0: din("xns", [T_MAX, NCONV, D], u8),
            1: din("xnp", [T_MAX, NCONV, D], u8)}
    wdefs = {}
    for sti, st in enumerate(("s", "p")):
        wdefs[sti] = dict(
            we=din(f"we_{st}", [D, 768], u8),
            wh=din(f"wh_{st}", [D, 1024], u8),
            wr=din(f"wr_{st}", [D, 1024], u8),
            b1=din(f"b1_{st}", [128, 6], f32) if with_bias else None,
            b2=din(f"b2_{st}", [128, 8], f32) if with_bias else None,
        )
    m8_d = din("m8", [NCONV, 128, 512], u8)   # 0/1 row-validity, per ut block
    outw_d = din("outw", [4 * D, C], u8)
    out_d = nc.dram_tensor("out", [NCONV, T_MAX, C], f32,
                           kind="ExternalOutput").ap()

    with ExitStack() as ctx:
        tc = ctx.enter_context(tile.TileContext(nc))
        const = ctx.enter_context(tc.tile_pool(name="const", bufs=1))
        xpool = ctx.enter_context(tc.tile_pool(name="xpool", bufs=12))
        work = ctx.enter_context(tc.tile_pool(name="work", bufs=2))
        fpool = ctx.enter_context(tc.tile_pool(name="fpool", bufs=3))
        opool = ctx.enter_context(tc.tile_pool(name="opool", bufs=2))
        gp = ctx.enter_context(tc.tile_pool(name="gp", bufs=2, space="PSUM"))
        ep = ctx.enter_context(tc.tile_pool(name="ep", bufs=2, space="PSUM"))

        # ---- constants / weights (tiles now; DMAs ordered by first use) --
        W = {}
        for sti, st in enumerate(("s", "p")):
            d = wdefs[sti]
            we_t = const.tile([128, 2, 768], fp8, name=f"we_t{st}")
            wh_t = const.tile([128, 2, 1024], fp8, name=f"wh_t{st}")
            wr_t = const.tile([128, 2, 1024], fp8, name=f"wr_t{st}")
            b1_t = b2_t = None
            if with_bias:
                b1_t = const.tile([128, 6], f32, name=f"b1_t{st}")
                b2_t = const.tile([128, 8], f32, name=f"b2_t{st}")
            W[sti] = dict(we=we_t, wh=wh_t, wr=wr_t, b1=b1_t, b2=b2_t)
        m8_t = const.tile([128, NCONV, 4, 128], fp8, name="m8_t")
        outw_t = const.tile([128, 8, C], fp8, name="outw_t")
        lg_all = const.tile([128, NCONV, 4, C], f32, name="lg_all")
        s_all = const.tile([128, NCONV * 4], f32, name="s_all")
        lnS = const.tile([128, NCONV * 4], f32, name="lnS")

        def emit_const_dma(phase):
            for sti, st in enumerate(("s", "p")):
                d, w = wdefs[sti], W[sti]
                if phase == 0:      # needed by F(0) immediately
                    nc.sync.dma_start(out=w["we"], in_=d["we"].bitcast(fp8)
                                      .rearrange("(kt p) m -> p kt m", p=128))
                    if with_bias:
                        nc.sync.dma_start(out=w["b1"], in_=d["b1"])
                        nc.sync.dma_start(out=w["b2"], in_=d["b2"])
                else:               # needed from B1(0)/B2(0) onward
                    nc.sync.dma_start(out=w["wh"], in_=d["wh"].bitcast(fp8)
                                      .rearrange("(kt p) m -> p kt m", p=128))
                    nc.sync.dma_start(out=w["wr"], in_=d["wr"].bitcast(fp8)
                                      .rearrange("(kt p) m -> p kt m", p=128))
            if phase == 1:
                nc.sync.dma_start(out=m8_t, in_=m8_d.bitcast(fp8).rearrange(
                    "j p (b c) -> p j b c", b=4))
                nc.sync.dma_start(out=outw_t, in_=outw_d.bitcast(fp8).rearrange(
                    "(kt p) c -> p kt c", p=128))

        def mm(ps, lhsT, rhs, start, stop, pm=PM):
            nc.tensor.matmul(ps, lhsT, rhs, start=start, stop=stop,
                             perf_mode=pm)

        def dims(j):
            Lv = int(slot_lens[j])
            # 16-aligned: DoubleRow LDWEIGHTS requires k-pair step % 16 == 0
            L = min(T_MAX, ((Lv + 15) // 16) * 16)
            UT = (Lv + 127) // 128
            return Lv, L, UT

        S = [None] * NCONV      # per-conv pipeline state

        def gate_act(pg_ap, out_ap, nsl, scale, bias_t, bcol):
            """[128,nsl,L] psum -> bf16; merged unless per-z biases needed."""
            if with_bias:
                for z in range(nsl):
                    nc.scalar.activation(out_ap[:, z, :], pg_ap[:, z, :],
                                         AF.Tanh, scale=scale,
                                         bias=bias_t[:, bcol + z:bcol + z + 1])
            else:
                nc.scalar.activation(out_ap, pg_ap, AF.Tanh, scale=scale)

        def emit_dma(j):
            Lv, L, UT = dims(j)
            LX = UT * 128
            xt_, xn_ = {}, {}
            for st in (0, 1):
                xt = xpool.tile([128, 2, LX], fp8, tag="xt", name=f"xt{j}_{st}")
                for kd in range(2):
                    nc.sync.dma_start(out=xt[:, kd, :],
                                      in_=xt_d[st].bitcast(fp8)[j, kd, :, 0:LX])
                xn = xpool.tile([128, 4, D], fp8, tag="xn", name=f"xn{j}_{st}")
                for ut in range(UT):
                    nc.sync.dma_start(
                        out=xn[:, ut, :],
                        in_=xn_d[st].bitcast(fp8)[ut * 128:(ut + 1) * 128, j, :])
                xt_[st], xn_[st] = xt, xn
            S[j] = dict(xt=xt_, xn=xn_)

        def gen_F(j):
            Lv, L, UT = dims(j)
            st_ = S[j]
            g1_ = {}
            for st in (0, 1):
                w = W[st]
                gts = []
                for gi in range(3):            # (i0,i1) (g0,g1) (o0,o1)
                    pg_t = gp.tile([128, 2, T_MAX], f32, tag="pg",
                                   name=f"pg1{j}_{st}_{gi}")
                    for z in range(2):
                        m = 2 * gi + z
                        mm(pg_t[:, z, 0:L], w["we"][:, :, m * 128:(m + 1) * 128],
                           st_['xt'][st][:, :, 0:L], True, True)
                    gt = work.tile([128, 2, L], bf16, tag="g1", bufs=10,
                                   name=f"g1{j}_{st}_{gi}")
                    gate_act(pg_t[:, :, 0:L], gt[:, :, :], 2, 0.5 / ws_e[st],
                             w["b1"], 2 * gi)
                    gts.append(gt)
                g1_[st] = gts
                yield
            cs1 = work.tile([128, 4, L], bf16, tag="cs", bufs=8, name=f"cs1{j}")
            for st in (0, 1):
                nc.vector.scalar_tensor_tensor(cs1[:, 2 * st:2 * st + 2, :],
                                               g1_[st][0][:, :, :], 1.0,
                                               g1_[st][1][:, :, :],
                                               ALU.add, ALU.mult)
            th1 = work.tile([128, 4, L], bf16, tag="th", bufs=3, name=f"th1{j}")
            nc.scalar.activation(th1[:, :, :], cs1[:, :, :], AF.Tanh, scale=0.5)
            hs1_ = {}
            for st in (0, 1):
                hs1 = work.tile([128, 2, L], fp8, tag="hs", bufs=14,
                                name=f"hs1{j}_{st}")
                nc.vector.scalar_tensor_tensor(hs1[:, :, :],
                                               g1_[st][2][:, :, :], 1.0,
                                               th1[:, 2 * st:2 * st + 2, :],
                                               ALU.add, ALU.mult)
                hs1_[st] = hs1
            st_.update(cs1=cs1, hs1=hs1_)

        def attention(j, st, hs_tile, step, L, UT):
            """A = exp(0.5*e); Z = 1/(m8 . A) — masking via m8/zeroed-xn."""
            xt = S[j]['xt'][st]
            A = work.tile([128, 4, L], fp8, tag="A", bufs=4,
                          name=f"A{j}_{st}_{step}")
            done = 0
            while done < UT:
                take = 2 if UT - done >= 2 else 1
                et = ep.tile([128, 2, T_MAX], f32, tag="e",
                             name=f"e{j}_{st}_{step}_{done}")
                for q in range(take):
                    ut = done + q
                    mm(et[:, q, 0:L], xt[:, :, ut * 128:(ut + 1) * 128],
                       hs_tile[:, :, :], True, True)
                nc.scalar.activation(A[:, done:done + take, :],
                                     et[:, 0:take, 0:L], AF.Exp, scale=0.5)
                done += take
            NPAIR, ODD = UT // 2, UT % 2
            pt = ep.tile([128, 2, T_MAX], f32, tag="e", name=f"ps{j}_{st}_{step}")
            for k in range(NPAIR):
                mm(pt[:, 0, 0:L], m8_t[:, j, 2 * k:2 * k + 2, :],
                   A[:, 2 * k:2 * k + 2, :], k == 0,
                   k == NPAIR - 1 and not ODD)
            if ODD:
                mm(pt[:, 0, 0:L], m8_t[:, j, UT - 1, :], A[:, UT - 1, :],
                   NPAIR == 0, True, pm=None)
            Z = work.tile([128, L], f32, tag="Z", bufs=4, name=f"Z{j}_{st}_{step}")
            nc.vector.reciprocal_approx_fast(Z[:, :], pt[:, 0, 0:L])
            return A, Z

        def r_apply(j, st, A, Z, out_tile, zoff, relu, step, L, UT):
            """out[:, zoff+dt, :] = (X^T A) * Z, optionally relu'd."""
            xn = S[j]['xn'][st]
            NPAIR, ODD = UT // 2, UT % 2
            for dt in range(2):
                rt = ep.tile([128, 2, T_MAX], f32, tag="e",
                             name=f"r{j}_{st}_{step}_{dt}")
                for k in range(NPAIR):
                    mm(rt[:, 0, 0:L],
                       xn[:, 2 * k:2 * k + 2, dt * 128:(dt + 1) * 128],
                       A[:, 2 * k:2 * k + 2, :], k == 0,
                       k == NPAIR - 1 and not ODD)
                if ODD:
                    mm(rt[:, 0, 0:L], xn[:, UT - 1, dt * 128:(dt + 1) * 128],
                       A[:, UT - 1, :], NPAIR == 0, True, pm=None)
                nc.vector.scalar_tensor_tensor(
                    out_tile[:, zoff + dt, :], rt[:, 0, 0:L],
                    0.0 if relu else 1.0, Z[:, :],
                    ALU.max if relu else ALU.mult, ALU.mult)

        def gen_B1(j):
            Lv, L, UT = dims(j)
            st_ = S[j]
            AZ1, r1_ = {}, {}
            for st in (0, 1):
                AZ1[st] = attention(j, st, st_['hs1'][st], 1, L, UT)
                yield
            for st in (0, 1):
                r1 = work.tile([128, 2, L], fp8, tag="r1", bufs=8,
                               name=f"r1{j}_{st}")
                r_apply(j, st, AZ1[st][0], AZ1[st][1], r1, 0, False, 1, L, UT)
                r1_[st] = r1
                yield
            st_.update(r1=r1_)

        def gen_B2(j):
            Lv, L, UT = dims(j)
            st_ = S[j]
            g2_ = {}
            for st in (0, 1):
                w = W[st]
                gts = []
                for gi in range(4):            # i, f, g, o pairs
                    pg_t = gp.tile([128, 2, T_MAX], f32, tag="pg",
                                   name=f"pg2{j}_{st}_{gi}")
                    for z in range(2):
                        m = 2 * gi + z
                        mm(pg_t[:, z, 0:L],
                           w["wh"][:, :, m * 128:(m + 1) * 128],
                           st_['hs1'][st][:, :, :], True, False)
                        mm(pg_t[:, z, 0:L],
                           w["wr"][:, :, m * 128:(m + 1) * 128],
                           st_['r1'][st][:, :, :], False, True)
                    gt = work.tile([128, 2, L], bf16, tag="g2", bufs=10,
                                   name=f"g2{j}_{st}_{gi}")
                    gate_act(pg_t[:, :, 0:L], gt[:, :, :], 2, 0.5 / ws_h[st],
                             w["b2"], 2 * gi)
                    gts.append(gt)
                g2_[st] = gts                  # [i, f, g, o]
                yield
            cs2 = work.tile([128, 4, L], bf16, tag="cs", bufs=8, name=f"cs2{j}")
            for st in (0, 1):
                gi2, gf2, gg2, go2 = g2_[st]
                t1 = work.tile([128, 2, L], bf16, tag="tmp", bufs=4,
                               name=f"t1{j}_{st}")
                nc.vector.scalar_tensor_tensor(t1[:, :, :], gf2[:, :, :], 1.0,
                                               st_['cs1'][:, 2 * st:2 * st + 2, :],
                                               ALU.add, ALU.mult)
                t2 = work.tile([128, 2, L], bf16, tag="tmp", bufs=4,
                               name=f"t2{j}_{st}")
                nc.vector.scalar_tensor_tensor(t2[:, :, :], gi2[:, :, :], 1.0,
                                               gg2[:, :, :], ALU.add, ALU.mult)
                nc.vector.scalar_tensor_tensor(cs2[:, 2 * st:2 * st + 2, :],
                                               t1[:, :, :], 0.5, t2[:, :, :],
                                               ALU.mult, ALU.add)
            th2 = work.tile([128, 4, L], bf16, tag="th", bufs=3, name=f"th2{j}")
            nc.scalar.activation(th2[:, :, :], cs2[:, :, :], AF.Tanh, scale=0.5)
            hs2_ = {}
            for st in (0, 1):
                hs2 = work.tile([128, 2, L], fp8, tag="hs", bufs=14,
                                name=f"hs2{j}_{st}")
                nc.vector.scalar_tensor_tensor(hs2[:, :, :],
                                               g2_[st][3][:, :, :], 1.0,
                                               th2[:, 2 * st:2 * st + 2, :],
                                               ALU.add, ALU.mult)
                hs2_[st] = hs2
            st_.update(hs2=hs2_)

        def gen_B3(j):
            Lv, L, UT = dims(j)
            st_ = S[j]
            ft_ = {}
            for st in (0, 1):
                A2, Z2 = attention(j, st, st_['hs2'][st], 2, L, UT)
                yield
                ft = fpool.tile([128, 4, L], fp8, tag=f"ft{st}", name=f"ft{j}_{st}")
                nc.vector.tensor_scalar_max(ft[:, 0:2, :],
                                            st_['hs2'][st][:, :, :], 0.0)
                r_apply(j, st, A2, Z2, ft, 2, True, 2, L, UT)
                ft_[st] = ft
                yield
            # logits transposed: [t, C] per 128-t chunk (8-wide slots in psum)
            lpt = ep.tile([128, 2, T_MAX], f32, tag="e", name=f"lp{j}")
            for cch in range(UT):
                ncch = min(128, L - cch * 128)
                for m, (ftile, zz) in enumerate(
                        ((ft_[0], 0), (ft_[0], 2), (ft_[1], 0), (ft_[1], 2))):
                    mm(lpt[0:ncch, 0, cch * 8:cch * 8 + C],
                       ftile[:, zz:zz + 2, cch * 128:cch * 128 + ncch],
                       outw_t[:, 2 * m:2 * m + 2, :], m == 0, m == 3)
            lgv = lpt[:, 0, 0:UT * 8].rearrange("p (u c) -> p u c", c=8)[:, :, 0:C]
            nc.vector.tensor_scalar_mul(lg_all[:, j, 0:UT, :], lgv, 1.0 / ows)
            elg = work.tile([128, 4, C], f32, tag="elg", name=f"elg{j}")
            nc.scalar.activation(elg[:, 0:UT, :], lgv, AF.Exp, scale=1.0 / ows)
            nc.vector.tensor_reduce(s_all[:, j * 4:j * 4 + UT],
                                    elg[:, 0:UT, :], AX.X, ALU.add)
            S[j] = None

        # ---- 4-stage software-pipelined main loop ----------------------
        emit_const_dma(0)
        emit_dma(0)
        emit_dma(1)
        emit_const_dma(1)
        for g in gen_F(0):
            pass
        for t in range(NCONV + 3):
            if t + 2 < NCONV:
                emit_dma(t + 2)
            gens = []
            if t + 1 < NCONV:
                gens.append(gen_F(t + 1))
            if t >= 1 and t - 1 < NCONV:
                gens.append(gen_B1(t - 1))
            if t >= 2 and t - 2 < NCONV:
                gens.append(gen_B2(t - 2))
            if t >= 3:
                gens.append(gen_B3(t - 3))
            while gens:
                nxt = []
                for g in gens:
                    try:
                        next(g)
                        nxt.append(g)
                    except StopIteration:
                        pass
                gens = nxt

        # ---- final: logp = lg - ln(rowsum) ----------------------------
        nc.scalar.activation(lnS[:, :], s_all[:, :], AF.Ln)
        for j in range(NCONV):
            UT = dims(j)[2]
            ot = opool.tile([128, 4, C], f32, tag="ot", name=f"ot{j}")
            for cch in range(UT):
                nc.vector.tensor_scalar_sub(ot[:, cch, :], lg_all[:, j, cch, :],
                                            lnS[:, j * 4 + cch:j * 4 + cch + 1])
            nc.sync.dma_start(
                out=out_d[j].rearrange("(c p) k -> p c k", p=128)[:, 0:UT, :],
                in_=ot[:, 0:UT, :])

    nc.compile()
    return nc


def _host_prep(inputs):
    """Fold weights, quantize to fp8, pick conversation->core assignment."""
    x_s = np.asarray(inputs["input"], dtype=np.float32)
    x_p = np.asarray(inputs["speakers"], dtype=np.float32)
    lengths = np.asarray(inputs["utterance_lengths"]).astype(np.int64)
    fc_w = np.asarray(inputs["fc_w"], dtype=np.float32)
    fc_b = np.asarray(inputs["fc_b"], dtype=np.float32)
    out_w = np.asarray(inputs["out_w"], dtype=np.float32)
    out_b = np.asarray(inputs["out_b"], dtype=np.float32)

    per_stream = {}
    scales = {}
    any_b = False
    for st in ("s", "p"):
        w_ih = np.asarray(inputs[f"w_ih_{st}"], dtype=np.float32)
        w_hh = np.asarray(inputs[f"w_hh_{st}"], dtype=np.float32)
        b_ih = np.asarray(inputs[f"b_ih_{st}"], dtype=np.float32)
        b_hh = np.asarray(inputs[f"b_hh_{st}"], dtype=np.float32)
        W_eff = w_ih @ fc_w                          # [1024, 256]
        bias1 = w_ih @ fc_b + b_ih + b_hh            # [1024]
        sel = np.r_[0:D, 2 * D:4 * D]                # i, g, o rows
        We = np.ascontiguousarray(W_eff[sel].T)      # [256, 768]
        We[:, D:2 * D] *= 2.0                        # g-gate doubling
        Wh = np.ascontiguousarray((0.5 * (w_ih[:, :D] + w_hh)).T)  # [256, 1024]
        Wr = np.ascontiguousarray(w_ih[:, D:].T)     # [256, 1024]
        Wh[:, 2 * D:3 * D] *= 2.0
        Wr[:, 2 * D:3 * D] *= 2.0
        ws_e = _pow2_scale(We)
        ws_h = _pow2_scale(np.concatenate([Wh, Wr], axis=0))
        scales[f'ws_e_{st}'] = ws_e
        scales[f'ws_h_{st}'] = ws_h
        # per-slice activation biases (pre-multiplied by the tanh input
        # scale: 0.5 normally, 1.0 for the doubled g-gate)
        b1_sel = bias1[sel]                          # [768] i,g,o
        bias2 = b_ih + b_hh                          # [1024] i,f,g,o
        b1_cols = np.zeros((128, 6), np.float32)
        for m in range(6):
            f = 1.0 if m in (2, 3) else 0.5
            b1_cols[:, m] = f * b1_sel[m * 128:(m + 1) * 128]
        b2_cols = np.zeros((128, 8), np.float32)
        for m in range(8):
            f = 1.0 if m in (4, 5) else 0.5
            b2_cols[:, m] = f * bias2[m * 128:(m + 1) * 128]
        any_b |= bool(np.any(b1_cols != 0.0) or np.any(b2_cols != 0.0))
        per_stream[st] = (_f8(We * ws_e), _f8(Wh * ws_h), _f8(Wr * ws_h),
                          b1_cols, b2_cols)

    # out_w columns for the h-halves get the 0.5 compensation (h stored as 2h)
    ow = out_w.copy()
    ow[:, 0:D] *= 0.5
    ow[:, 2 * D:3 * D] *= 0.5
    ows = _pow2_scale(ow)
    scales['ows'] = ows
    outw8 = _f8(ow.T * ows)                          # [1024, 7]
    host_out_b = out_b

    # conversation -> (core, slot): sort by length desc, round-robin
    order = np.argsort(-lengths, kind="stable")
    assign = {}
    for rank, conv in enumerate(order):
        assign[int(conv)] = (rank % NCORE, rank // NCORE)
    order_lens = lengths[order]
    slot_lens = tuple(int(order_lens[8 * k]) for k in range(NCONV))

    # fp8-quantize the banks once (identical bytes for both layouts)
    import ml_dtypes
    xs8 = np.clip(x_s, -240.0, 240.0).astype(ml_dtypes.float8_e4m3fn).view(np.uint8)
    xp8 = np.clip(x_p, -240.0, 240.0).astype(ml_dtypes.float8_e4m3fn).view(np.uint8)
    one8 = int(np.array([1.0], dtype=ml_dtypes.float8_e4m3fn).view(np.uint8)[0])

    in_maps = []
    core_convs = []
    for core in range(NCORE):
        ids = [None] * NCONV
        for conv, (c, s) in assign.items():
            if c == core:
                ids[s] = conv
        core_convs.append(ids)
        m8 = np.zeros((NCONV, 128, 512), dtype=np.uint8)
        xns = xs8[:, ids, :].copy()      # [T_MAX, NCONV, D], u-major
        xnp = xp8[:, ids, :].copy()
        for s, conv in enumerate(ids):
            Lc = int(lengths[conv])
            valid = (np.arange(T_MAX) < Lc)
            m8[s, :, :] = np.where(valid, one8, 0).astype(np.uint8).reshape(
                4, 128).T.repeat(128, axis=1).reshape(128, 512)
            xns[Lc:, s, :] = 0
            xnp[Lc:, s, :] = 0
        im = {
            "xts": np.ascontiguousarray(
                xs8[:, ids, :].transpose(1, 2, 0).reshape(NCONV, 2, 128, T_MAX)),
            "xtp": np.ascontiguousarray(
                xp8[:, ids, :].transpose(1, 2, 0).reshape(NCONV, 2, 128, T_MAX)),
            "xns": np.ascontiguousarray(xns),
            "xnp": np.ascontiguousarray(xnp),
            "m8": m8,
            "outw": outw8,
        }
        for st in ("s", "p"):
            We8, Wh8, Wr8, b1c, b2c = per_stream[st]
            im[f"we_{st}"] = We8
            im[f"wh_{st}"] = Wh8
            im[f"wr_{st}"] = Wr8
            if any_b:
                im[f"b1_{st}"] = b1c
                im[f"b2_{st}"] = b2c
        in_maps.append(im)
    key = (any_b, slot_lens,
           tuple(sorted((k, float(v)) for k, v in scales.items())))
    return in_maps, core_convs, lengths, key, scales, host_out_b


def _gather(results, core_convs, lengths, out_b):
    """results: per-core {'out': [NCONV, T_MAX, C]} -> [sum(len), C]."""
    where = {}
    for core, ids in enumerate(core_convs):
        for slot, conv in enumerate(ids):
            where[conv] = (core, slot)
    chunks = []
    nz = bool(np.any(out_b != 0.0))
    for b in range(BATCH):
        core, slot = where[b]
        L = int(lengths[b])
        lg = results[core]["out"][slot, :L, :]
        if nz:
            # device log-softmax omitted out_b; log_softmax is shift-invariant
            # per row, so redo it with the bias added.
            lg = lg + out_b[None, :]
            lg = lg - np.log(np.exp(lg).sum(axis=1, keepdims=True))
        chunks.append(np.ascontiguousarray(lg))
    return np.concatenate(chunks, axis=0).astype(np.float32)


def _get_nc(key, scales):
    if key not in _BUILD_CACHE:
        _BUILD_CACHE[key] = _build(key[0], key[1], scales)
    return _BUILD_CACHE[key]


def kernel(**inputs):
    from concourse import bass_utils
    in_maps, core_convs, lengths, key, scales, out_b = _host_prep(inputs)
    nc = _get_nc(key, scales)
    res = bass_utils.run_bass_kernel_spmd(nc, in_maps, core_ids=list(range(NCORE)))
    return _gather(res.results, core_convs, lengths, out_b)


# revision 32
# speedup vs baseline: 1.0304x; 1.0071x over previous
"""DCRNCognition Trainium2 kernel — fp8 DoubleRow edition.

Self-contained: builds a Bass/Tile SPMD program for 8 NeuronCores, shards the
batch (conversation) axis across cores, runs via run_bass_kernel_spmd, and
gathers the valid positions on the host.

Math restructuring (identical to the verified baseline, rel err ~9e-7 in f32):
  - fc layer folded into step-1 LSTM gates; step-1 f-gate/c-init dead
  - step-2: gates2 = hs1 @ Wh.T + r1 @ Wr.T  (Wh = 0.5*(w_ih[:, :D]+w_hh))
  - softmax normalization deferred to r:  r = (X^T A) * (1/sum_u A)
  - sigmoid via tanh; h,c carried scaled by 2 (hs=2h, cs=2c)

Precision plan (validated on host: rel err ~6e-4 vs the 2e-2 gate):
  - ALL matmuls fp8 e4m3 with DoubleRow perf mode, fp32 PSUM accum.
  - weights pre-scaled by a power of 2 into fp8 range on the host; the
    compensation folds into the free activation `scale` operands.
  - g-gate weight rows doubled on host so every gate activation shares
    scale=0.5 -> one ACT instruction per gate pair.
  - masking without exp bias: invalid bank rows (u >= len) are zeroed on
    the host in the u-major layout (kills their r contribution) and the
    softmax denominator contracts A against a per-conversation 0/1 mask
    as the matmul stationary (kills them in the sum).
  - logits computed transposed ([t,C]): log-softmax along the free axis,
    one batched Ln at the end -> 2 ACT table loads total.

Scheduling: 4-stage software pipeline (F=gates1+cell1, B1=attention1,
B2=gates2+cell2, B3=attention2+logits) with a 3-conversation skew so every
engine queue always holds ready work. PSUM: two 4-bank pools (gates /
attention+logits).
"""
import os
import sys
sys.path.insert(0, '/opt/trn_rl_repo')

# run_bass_kernel_spmd executes through jax/PJRT on the axon-tunneled
# NeuronCores; a JAX_PLATFORMS=cpu pin would hide them.
if os.environ.get('JAX_PLATFORMS') == 'cpu' and 'jax' not in sys.modules:
    del os.environ['JAX_PLATFORMS']

import math
import numpy as np

T_MAX, BATCH, D, C = 512, 128, 256, 7
NCORE = 8
NCONV = BATCH // NCORE          # conversations per core

_BUILD_CACHE = {}


def _f8(x):
    """Host fp32 -> e4m3 bytes (clipped to the TRN-compatible +-240 range)."""
    import ml_dtypes
    return np.ascontiguousarray(
        np.clip(np.asarray(x, np.float32), -240.0, 240.0)
        .astype(ml_dtypes.float8_e4m3fn).view(np.uint8))


def _pow2_scale(w):
    s = float(np.std(w))
    if s == 0.0 or not np.isfinite(s):
        return 1.0
    return float(2.0 ** round(math.log2(4.0 / s)))


def _build(with_bias, slot_lens, scales):
    """Build + compile the SPMD Bass program. Returns the Bacc instance."""
    from contextlib import ExitStack
    import concourse.bacc as bacc
    import concourse.bass as bass  # noqa: F401
    from concourse import mybir, tile

    f32 = mybir.dt.float32
    bf16 = mybir.dt.bfloat16
    fp8 = mybir.dt.float8e4
    u8 = mybir.dt.uint8
    AF = mybir.ActivationFunctionType
    ALU = mybir.AluOpType
    AX = mybir.AxisListType
    PM = mybir.MatmulPerfMode.DoubleRow

    ws_e = {0: scales['ws_e_s'], 1: scales['ws_e_p']}
    ws_h = {0: scales['ws_h_s'], 1: scales['ws_h_p']}
    ows = scales['ows']

    nc = bacc.Bacc("TRN2", target_bir_lowering=False, debug=False,
                   num_devices=NCORE)

    def din(name, shape, dt):
        return nc.dram_tensor(name, shape, dt, kind="ExternalInput").ap()

    xt_d = {0: din("xts", [NCONV, 2, 128, T_MAX], u8),
            1: din("xtp", [NCONV, 2, 128, T_MAX], u8)}
    xn_d = {0: din("xns", [T_MAX, NCONV, D], u8),
            1: din("xnp", [T_MAX, NCONV, D], u8)}
    wdefs = {}
    for sti, st in enumerate(("s", "p")):
        wdefs[sti] = dict(
            we=din(f"we_{st}", [D, 768], u8),
            wh=din(f"wh_{st}", [D, 1024], u8),
            wr=din(f"wr_{st}", [D, 1024], u8),
            b1=din(f"b1_{st}", [128, 6], f32) if with_bias else None,
            b2=din(f"b2_{st}", [128, 8], f32) if with_bias else None,
        )
    m8_d = din("m8", [NCONV, 128, 512], u8)   # 0/1 row-validity, per ut block
    outw_d = din("outw", [4 * D, C], u8)
    out_d = nc.dram_tensor("out", [NCONV, T_MAX, C], f32,
                           kind="ExternalOutput").ap()

    with ExitStack() as ctx:
        tc = ctx.enter_context(tile.TileContext(nc))
        const = ctx.enter_context(tc.tile_pool(name="const", bufs=1))
        xpool = ctx.enter_context(tc.tile_pool(name="xpool", bufs=12))
        work = ctx.enter_context(tc.tile_pool(name="work", bufs=2))
        fpool = ctx.enter_context(tc.tile_pool(name="fpool", bufs=3))
        opool = ctx.enter_context(tc.tile_pool(name="opool", bufs=2))
        gp = ctx.enter_context(tc.tile_pool(name="gp", bufs=2, space="PSUM"))
        ep = ctx.enter_context(tc.tile_pool(name="ep", bufs=2, space="PSUM"))

        # ---- constants / weights (tiles now; DMAs ordered by first use) --
        W = {}
        for sti, st in enumerate(("s", "p")):
            d = wdefs[sti]
            we_t = const.tile([128, 2, 768], fp8, name=f"we_t{st}")
            wh_t = const.tile([128, 2, 1024], fp8, name=f"wh_t{st}")
            wr_t = const.tile([128, 2, 1024], fp8, name=f"wr_t{st}")
            b1_t = b2_t = None
            if with_bias:
                b1_t = const.tile([128, 6], f32, name=f"b1_t{st}")
                b2_t = const.tile([128, 8], f32, name=f"b2_t{st}")
            W[sti] = dict(we=we_t, wh=wh_t, wr=wr_t, b1=b1_t, b2=b2_t)
        m8_t = const.tile([128, NCONV, 4, 128], fp8, name="m8_t")
        outw_t = const.tile([128, 8, C], fp8, name="outw_t")
        lg_all = const.tile([128, NCONV, 4, C], f32, name="lg_all")
        s_all = const.tile([128, NCONV * 4], f32, name="s_all")
        lnS = const.tile([128, NCONV * 4], f32, name="lnS")

        def emit_const_dma(phase):
            for sti, st in enumerate(("s", "p")):
                d, w = wdefs[sti], W[sti]
                if phase == 0:      # needed by F(0) immediately
                    nc.sync.dma_start(out=w["we"], in_=d["we"].bitcast(fp8)
                                      .rearrange("(kt p) m -> p kt m", p=128))
                    if with_bias:
                        nc.sync.dma_start(out=w["b1"], in_=d["b1"])
                        nc.sync.dma_start(out=w["b2"], in_=d["b2"])
                else:               # needed from B1(0)/B2(0) onward
                    nc.sync.dma_start(out=w["wh"], in_=d["wh"].bitcast(fp8)
                                      .rearrange("(kt p) m -> p kt m", p=128))
                    nc.sync.dma_start(out=w["wr"], in_=d["wr"].bitcast(fp8)
                                      .rearrange("(kt p) m -> p kt m", p=128))
            if phase == 1:
                nc.sync.dma_start(out=m8_t, in_=m8_d.bitcast(fp8).rearrange(
                    "j p (b c) -> p j b c", b=4))
                nc.sync.dma_start(out=outw_t, in_=outw_d.bitcast(fp8).rearrange(
                    "(kt p) c -> p kt c", p=128))

        def mm(ps, lhsT, rhs, start, stop, pm=PM):
            nc.tensor.matmul(ps, lhsT, rhs, start=start, stop=stop,
                             perf_mode=pm)

        def dims(j):
            Lv = int(slot_lens[j])
            # 16-aligned: DoubleRow LDWEIGHTS requires k-pair step % 16 == 0
            L = min(T_MAX, ((Lv + 15) // 16) * 16)
            UT = (Lv + 127) // 128
            return Lv, L, UT

        S = [None] * NCONV      # per-conv pipeline state

        def gate_act(pg_ap, out_ap, nsl, scale, bias_t, bcol):
            """[128,nsl,L] psum -> bf16; merged unless per-z biases needed."""
            if with_bias:
                for z in range(nsl):
                    nc.scalar.activation(out_ap[:, z, :], pg_ap[:, z, :],
                                         AF.Tanh, scale=scale,
                                         bias=bias_t[:, bcol + z:bcol + z + 1])
            else:
                nc.scalar.activation(out_ap, pg_ap, AF.Tanh, scale=scale)

        def emit_dma(j):
            Lv, L, UT = dims(j)
            LX = UT * 128
            xt_, xn_ = {}, {}
            for st in (0, 1):
                xt = xpool.tile([128, 2, LX], fp8, tag="xt", name=f"xt{j}_{st}")
                for kd in range(2):
                    nc.sync.dma_start(out=xt[:, kd, :],
                                      in_=xt_d[st].bitcast(fp8)[j, kd, :, 0:LX])
                xn = xpool.tile([128, 4, D], fp8, tag="xn", name=f"xn{j}_{st}")
                for ut in range(UT):
                    nc.sync.dma_start(
                        out=xn[:, ut, :],
                        in_=xn_d[st].bitcast(fp8)[ut * 128:(ut + 1) * 128, j, :])
                xt_[st], xn_[st] = xt, xn
            S[j] = dict(xt=xt_, xn=xn_)

        def gen_F(j):
            Lv, L, UT = dims(j)
            st_ = S[j]
            g1_ = {}
            for st in (0, 1):
                w = W[st]
                gts = []
                for gi in range(3):            # (i0,i1) (g0,g1) (o0,o1)
                    pg_t = gp.tile([128, 2, T_MAX], f32, tag="pg",
                                   name=f"pg1{j}_{st}_{gi}")
                    for z in range(2):
                        m = 2 * gi + z
                        mm(pg_t[:, z, 0:L], w["we"][:, :, m * 128:(m + 1) * 128],
                           st_['xt'][st][:, :, 0:L], True, True)
                    gt = work.tile([128, 2, L], bf16, tag="g1", bufs=10,
                                   name=f"g1{j}_{st}_{gi}")
                    gate_act(pg_t[:, :, 0:L], gt[:, :, :], 2, 0.5 / ws_e[st],
                             w["b1"], 2 * gi)
                    gts.append(gt)
                g1_[st] = gts
                yield
            cs1 = work.tile([128, 4, L], bf16, tag="cs", bufs=8, name=f"cs1{j}")
            for st in (0, 1):
                nc.vector.scalar_tensor_tensor(cs1[:, 2 * st:2 * st + 2, :],
                                               g1_[st][0][:, :, :], 1.0,
                                               g1_[st][1][:, :, :],
                                               ALU.add, ALU.mult)
            th1 = work.tile([128, 4, L], bf16, tag="th", bufs=3, name=f"th1{j}")
            nc.scalar.activation(th1[:, :, :], cs1[:, :, :], AF.Tanh, scale=0.5)
            hs1_ = {}
            for st in (0, 1):
                hs1 = work.tile([128, 2, L], fp8, tag="hs", bufs=14,
                                name=f"hs1{j}_{st}")
                nc.vector.scalar_tensor_tensor(hs1[:, :, :],
                                               g1_[st][2][:, :, :], 1.0,
                                               th1[:, 2 * st:2 * st + 2, :],
                                               ALU.add, ALU.mult)
                hs1_[st] = hs1
            st_.update(cs1=cs1, hs1=hs1_)

        def attention(j, st, hs_tile, step, L, UT):
            """A = exp(0.5*e); Z = 1/(m8 . A) — masking via m8/zeroed-xn."""
            xt = S[j]['xt'][st]
            A = work.tile([128, 4, L], fp8, tag="A", bufs=4,
                          name=f"A{j}_{st}_{step}")
            done = 0
            while done < UT:
                take = 2 if UT - done >= 2 else 1
                et = ep.tile([128, 2, T_MAX], f32, tag="e",
                             name=f"e{j}_{st}_{step}_{done}")
                for q in range(take):
                    ut = done + q
                    mm(et[:, q, 0:L], xt[:, :, ut * 128:(ut + 1) * 128],
                       hs_tile[:, :, :], True, True)
                nc.scalar.activation(A[:, done:done + take, :],
                                     et[:, 0:take, 0:L], AF.Exp, scale=0.5)
                done += take
            NPAIR, ODD = UT // 2, UT % 2
            pt = ep.tile([128, 2, T_MAX], f32, tag="e", name=f"ps{j}_{st}_{step}")
            for k in range(NPAIR):
                mm(pt[:, 0, 0:L], m8_t[:, j, 2 * k:2 * k + 2, :],
                   A[:, 2 * k:2 * k + 2, :], k == 0,
                   k == NPAIR - 1 and not ODD)
            if ODD:
                mm(pt[:, 0, 0:L], m8_t[:, j, UT - 1, :], A[:, UT - 1, :],
                   NPAIR == 0, True, pm=None)
            Z = work.tile([128, L], f32, tag="Z", bufs=4, name=f"Z{j}_{st}_{step}")
            nc.vector.reciprocal_approx_fast(Z[:, :], pt[:, 0, 0:L])
            return A, Z

        def r_apply(j, st, A, Z, out_tile, zoff, relu, step, L, UT):
            """out[:, zoff+dt, :] = (X^T A) * Z, optionally relu'd."""
            xn = S[j]['xn'][st]
            NPAIR, ODD = UT // 2, UT % 2
            for dt in range(2):
                rt = ep.tile([128, 2, T_MAX], f32, tag="e",
                             name=f"r{j}_{st}_{step}_{dt}")
                for k in range(NPAIR):
                    mm(rt[:, 0, 0:L],
                       xn[:, 2 * k:2 * k + 2, dt * 128:(dt + 1) * 128],
                       A[:, 2 * k:2 * k + 2, :], k == 0,
                       k == NPAIR - 1 and not ODD)
                if ODD:
                    mm(rt[:, 0, 0:L], xn[:, UT - 1, dt * 128:(dt + 1) * 128],
                       A[:, UT - 1, :], NPAIR == 0, True, pm=None)
                nc.vector.scalar_tensor_tensor(
                    out_tile[:, zoff + dt, :], rt[:, 0, 0:L],
                    0.0 if relu else 1.0, Z[:, :],
                    ALU.max if relu else ALU.mult, ALU.mult)

        def gen_B1(j):
            Lv, L, UT = dims(j)
            st_ = S[j]
            AZ1, r1_ = {}, {}
            for st in (0, 1):
                AZ1[st] = attention(j, st, st_['hs1'][st], 1, L, UT)
                yield
            for st in (0, 1):
                r1 = work.tile([128, 2, L], fp8, tag="r1", bufs=8,
                               name=f"r1{j}_{st}")
                r_apply(j, st, AZ1[st][0], AZ1[st][1], r1, 0, False, 1, L, UT)
                r1_[st] = r1
                yield
            st_.update(r1=r1_)

        def gen_B2(j):
            Lv, L, UT = dims(j)
            st_ = S[j]
            g2_ = {}
            for st in (0, 1):
                w = W[st]
                gts = []
                for gi in range(4):            # i, f, g, o pairs
                    pg_t = gp.tile([128, 2, T_MAX], f32, tag="pg",
                                   name=f"pg2{j}_{st}_{gi}")
                    for z in range(2):
                        m = 2 * gi + z
                        mm(pg_t[:, z, 0:L],
                           w["wh"][:, :, m * 128:(m + 1) * 128],
                           st_['hs1'][st][:, :, :], True, False)
                        mm(pg_t[:, z, 0:L],
                           w["wr"][:, :, m * 128:(m + 1) * 128],
                           st_['r1'][st][:, :, :], False, True)
                    gt = work.tile([128, 2, L], bf16, tag="g2", bufs=10,
                                   name=f"g2{j}_{st}_{gi}")
                    gate_act(pg_t[:, :, 0:L], gt[:, :, :], 2, 0.5 / ws_h[st],
                             w["b2"], 2 * gi)
                    gts.append(gt)
                g2_[st] = gts                  # [i, f, g, o]
                yield
            cs2 = work.tile([128, 4, L], bf16, tag="cs", bufs=8, name=f"cs2{j}")
            for st in (0, 1):
                gi2, gf2, gg2, go2 = g2_[st]
                t1 = work.tile([128, 2, L], bf16, tag="tmp", bufs=4,
                               name=f"t1{j}_{st}")
                nc.vector.scalar_tensor_tensor(t1[:, :, :], gf2[:, :, :], 1.0,
                                               st_['cs1'][:, 2 * st:2 * st + 2, :],
                                               ALU.add, ALU.mult)
                t2 = work.tile([128, 2, L], bf16, tag="tmp", bufs=4,
                               name=f"t2{j}_{st}")
                nc.vector.scalar_tensor_tensor(t2[:, :, :], gi2[:, :, :], 1.0,
                                               gg2[:, :, :], ALU.add, ALU.mult)
                nc.vector.scalar_tensor_tensor(cs2[:, 2 * st:2 * st + 2, :],
                                               t1[:, :, :], 0.5, t2[:, :, :],
                                               ALU.mult, ALU.add)
            th2 = work.tile([128, 4, L], bf16, tag="th", bufs=3, name=f"th2{j}")
            nc.scalar.activation(th2[:, :, :], cs2[:, :, :], AF.Tanh, scale=0.5)
            hs2_ = {}
            for st in (0, 1):
                hs2 = work.tile([128, 2, L], fp8, tag="hs", bufs=14,
                                name=f"hs2{j}_{st}")
                nc.vector.scalar_tensor_tensor(hs2[:, :, :],
                                               g2_[st][3][:, :, :], 1.0,
                                               th2[:, 2 * st:2 * st + 2, :],
                                               ALU.add, ALU.mult)
                hs2_[st] = hs2
            st_.update(hs2=hs2_)

        def gen_B3(j):
            Lv, L, UT = dims(j)
            st_ = S[j]
            ft_ = {}
            for st in (0, 1):
                A2, Z2 = attention(j, st, st_['hs2'][st], 2, L, UT)
                yield
                ft = fpool.tile([128, 4, L], fp8, tag=f"ft{st}", name=f"ft{j}_{st}")
                nc.vector.tensor_scalar_max(ft[:, 0:2, :],
                                            st_['hs2'][st][:, :, :], 0.0)
                r_apply(j, st, A2, Z2, ft, 2, True, 2, L, UT)
                ft_[st] = ft
                yield
            # logits transposed: [t, C] per 128-t chunk (8-wide slots in psum)
            lpt = ep.tile([128, 2, T_MAX], f32, tag="e", name=f"lp{j}")
            for cch in range(UT):
                ncch = min(128, L - cch * 128)
                for m, (ftile, zz) in enumerate(
                        ((ft_[0], 0), (ft_[0], 2), (ft_[1], 0), (ft_[1], 2))):
                    mm(lpt[0:ncch, 0, cch * 8:cch * 8 + C],
                       ftile[:, zz:zz + 2, cch * 128:cch * 128 + ncch],
                       outw_t[:, 2 * m:2 * m + 2, :], m == 0, m == 3)
            lgv = lpt[:, 0, 0:UT * 8].rearrange("p (u c) -> p u c", c=8)[:, :, 0:C]
            nc.vector.tensor_scalar_mul(lg_all[:, j, 0:UT, :], lgv, 1.0 / ows)
            elg = work.tile([128, 4, C], f32, tag="elg", name=f"elg{j}")
            nc.scalar.activation(elg[:, 0:UT, :], lgv, AF.Exp, scale=1.0 / ows)
            nc.vector.tensor_reduce(s_all[:, j * 4:j * 4 + UT],
                                    elg[:, 0:UT, :], AX.X, ALU.add)
            S[j] = None

        # ---- 4-stage software-pipelined main loop ----------------------
        emit_const_dma(0)
        emit_dma(0)
        emit_dma(1)
        emit_const_dma(1)
        for g in gen_F(0):
            pass
        for t in range(NCONV + 3):
            if t + 2 < NCONV:
                emit_dma(t + 2)
            gens = []
            if t + 1 < NCONV:
                gens.append(gen_F(t + 1))
            if t >= 1 and t - 1 < NCONV:
                gens.append(gen_B1(t - 1))
            if t >= 2 and t - 2 < NCONV:
                gens.append(gen_B2(t - 2))
            if t >= 3:
                gens.append(gen_B3(t - 3))
            while gens:
                nxt = []
                for g in gens:
                    try:
                        next(g)
                        nxt.append(g)
                    except StopIteration:
                        pass
                gens = nxt

        # ---- final: logp = lg - ln(rowsum) ----------------------------
        nc.scalar.activation(lnS[:, :], s_all[:, :], AF.Ln)
        for j in range(NCONV):
            UT = dims(j)[2]
            ot = opool.tile([128, 4, C], f32, tag="ot", name=f"ot{j}")
            for cch in range(UT):
                nc.vector.tensor_scalar_sub(ot[:, cch, :], lg_all[:, j, cch, :],
                                            lnS[:, j * 4 + cch:j * 4 + cch + 1])
            nc.sync.dma_start(
                out=out_d[j].rearrange("(c p) k -> p c k", p=128)[:, 0:UT, :],
                in_=ot[:, 0:UT, :])

    nc.compile()
    return nc


def _host_prep(inputs):
    """Fold weights, quantize to fp8, pick conversation->core assignment."""
    x_s = np.asarray(inputs["input"], dtype=np.float32)
    x_p = np.asarray(inputs["speakers"], dtype=np.float32)
    lengths = np.asarray(inputs["utterance_lengths"]).astype(np.int64)
    fc_w = np.asarray(inputs["fc_w"], dtype=np.float32)
    fc_b = np.asarray(inputs["fc_b"], dtype=np.float32)
    out_w = np.asarray(inputs["out_w"], dtype=np.float32)
    out_b = np.asarray(inputs["out_b"], dtype=np.float32)

    per_stream = {}
    scales = {}
    any_b = False
    for st in ("s", "p"):
        w_ih = np.asarray(inputs[f"w_ih_{st}"], dtype=np.float32)
        w_hh = np.asarray(inputs[f"w_hh_{st}"], dtype=np.float32)
        b_ih = np.asarray(inputs[f"b_ih_{st}"], dtype=np.float32)
        b_hh = np.asarray(inputs[f"b_hh_{st}"], dtype=np.float32)
        W_eff = w_ih @ fc_w                          # [1024, 256]
        bias1 = w_ih @ fc_b + b_ih + b_hh            # [1024]
        sel = np.r_[0:D, 2 * D:4 * D]                # i, g, o rows
        We = np.ascontiguousarray(W_eff[sel].T)      # [256, 768]
        We[:, D:2 * D] *= 2.0                        # g-gate doubling
        Wh = np.ascontiguousarray((0.5 * (w_ih[:, :D] + w_hh)).T)  # [256, 1024]
        Wr = np.ascontiguousarray(w_ih[:, D:].T)     # [256, 1024]
        Wh[:, 2 * D:3 * D] *= 2.0
        Wr[:, 2 * D:3 * D] *= 2.0
        ws_e = _pow2_scale(We)
        ws_h = _pow2_scale(np.concatenate([Wh, Wr], axis=0))
        scales[f'ws_e_{st}'] = ws_e
        scales[f'ws_h_{st}'] = ws_h
        # per-slice activation biases (pre-multiplied by the tanh input
        # scale: 0.5 normally, 1.0 for the doubled g-gate)
        b1_sel = bias1[sel]                          # [768] i,g,o
        bias2 = b_ih + b_hh                          # [1024] i,f,g,o
        b1_cols = np.zeros((128, 6), np.float32)
        for m in range(6):
            f = 1.0 if m in (2, 3) else 0.5
            b1_cols[:, m] = f * b1_sel[m * 128:(m + 1) * 128]
        b2_cols = np.zeros((128, 8), np.float32)
        for m in range(8):
            f = 1.0 if m in (4, 5) else 0.5
            b2_cols[:, m] = f * bias2[m * 128:(m + 1) * 128]
        any_b |= bool(np.any(b1_cols != 0.0) or np.any(b2_cols != 0.0))
        per_stream[st] = (_f8(We * ws_e), _f8(Wh * ws_h), _f8(Wr * ws_h),
                          b1_cols, b2_cols)

    # out_w columns for the h-halves get the 0.5 compensation (h stored as 2h)
    ow = out_w.copy()
    ow[:, 0:D] *= 0.5
    ow[:, 2 * D:3 * D] *= 0.5
    ows = _pow2_scale(ow)
    scales['ows'] = ows
    outw8 = _f8(ow.T * ows)                          # [1024, 7]
    host_out_b = out_b

    # conversation -> (core, slot): sort by length desc, round-robin
    order = np.argsort(-lengths, kind="stable")
    assign = {}
    for rank, conv in enumerate(order):
        assign[int(conv)] = (rank % NCORE, rank // NCORE)
    order_lens = lengths[order]
    slot_lens = tuple(int(order_lens[8 * k]) for k in range(NCONV))

    # fp8-quantize the banks once (identical bytes for both layouts)
    import ml_dtypes
    xs8 = np.clip(x_s, -240.0, 240.0).astype(ml_dtypes.float8_e4m3fn).view(np.uint8)
    xp8 = np.clip(x_p, -240.0, 240.0).astype(ml_dtypes.float8_e4m3fn).view(np.uint8)
    one8 = int(np.array([1.0], dtype=ml_dtypes.float8_e4m3fn).view(np.uint8)[0])

    in_maps = []
    core_convs = []
    for core in range(NCORE):
        ids = [None] * NCONV
        for conv, (c, s) in assign.items():
            if c == core:
                ids[s] = conv
        core_convs.append(ids)
        m8 = np.zeros((NCONV, 128, 512), dtype=np.uint8)
        xns = xs8[:, ids, :].copy()      # [T_MAX, NCONV, D], u-major
        xnp = xp8[:, ids, :].copy()
        for s, conv in enumerate(ids):
            Lc = int(lengths[conv])
            valid = (np.arange(T_MAX) < Lc)
            m8[s, :, :] = np.where(valid, one8, 0).astype(np.uint8).reshape(
                4, 128).T.repeat(128, axis=1).reshape(128, 512)
            xns[Lc:, s, :] = 0
            xnp[Lc:, s, :] = 0
        im = {
            "xts": np.ascontiguousarray(
                xs8[:, ids, :].transpose(1, 2, 0).reshape(NCONV, 2, 128, T_MAX)),
            "xtp": np.ascontiguousarray(
                xp8[:, ids, :].transpose(1, 2, 0).reshape(NCONV, 2, 128, T_MAX)),
            "xns": np.ascontiguousarray(xns),
            "xnp": np.ascontiguousarray(xnp),
            "m8": m8,
            "outw": outw8,
        }
        for st in ("s", "p"):
            We8, Wh8, Wr8, b1c, b2c = per_stream[st]
            im[f"we_{st}"] = We8
            im[f"wh_{st}"] = Wh8
            im[f"wr_{st}"] = Wr8
            if any_b:
                im[f"b1_{st}"] = b1c
                im[f"b2_{st}"] = b2c
        in_maps.append(im)
    key = (any_b, slot_lens,
           tuple(sorted((k, float(v)) for k, v in scales.items())))
    return in_maps, core_convs, lengths, key, scales, host_out_b


def _gather(results, core_convs, lengths, out_b):
    """results: per-core {'out': [NCONV, T_MAX, C]} -> [sum(len), C]."""
    where = {}
    for core, ids in enumerate(core_convs):
        for slot, conv in enumerate(ids):
            where[conv] = (core, slot)
    chunks = []
    nz = bool(np.any(out_b != 0.0))
    for b in range(BATCH):
        core, slot = where[b]
        L = int(lengths[b])
        lg = results[core]["out"][slot, :L, :]
        if nz:
            # device log-softmax omitted out_b; log_softmax is shift-invariant
            # per row, so redo it with the bias added.
            lg = lg + out_b[None, :]
            lg = lg - np.log(np.exp(lg).sum(axis=1, keepdims=True))
        chunks.append(np.ascontiguousarray(lg))
    return np.concatenate(chunks, axis=0).astype(np.float32)


def _get_nc(key, scales):
    if key not in _BUILD_CACHE:
        _BUILD_CACHE[key] = _build(key[0], key[1], scales)
    return _BUILD_CACHE[key]


def kernel(**inputs):
    from concourse import bass_utils
    in_maps, core_convs, lengths, key, scales, out_b = _host_prep(inputs)
    nc = _get_nc(key, scales)
    res = bass_utils.run_bass_kernel_spmd(nc, in_maps, core_ids=list(range(NCORE)))
    return _gather(res.results, core_convs, lengths, out_b)
